# revision 36
# baseline (speedup 1.0000x reference)
"""Self-contained TRN2 Bass kernel for nn_GATRotationRegressor.

kernel(**inputs) -> [16384, 24, 6] fp32. Data-parallel over 8 NeuronCores;
all layouts/shapes hardcoded for B=16384, J=24, H=128, heads=4, L=3.

v2: GM-native xl+scores via per-chunk stationary matmuls (one transpose per
layer instead of three), LN stats via free-dim reduces in graph-major
layout, elementwise work balanced across Vector/GpSimd/Scalar engines.
"""
from contextlib import ExitStack

import numpy as np

import concourse.bass as bass
import concourse.tile as tile
from concourse import mybir


PARENTS = [-1, 0, 0, 0, 1, 2, 3, 4, 5, 6, 7, 8, 9, 9, 9, 12, 13, 14, 16, 17, 18, 19, 20, 21]
B, J, IN_DIM, H, HEADS, OUT_DIM, L = 16384, 24, 3, 128, 4, 6, 3
C = H // HEADS
SLOPE = 0.2
KMAX = 5           # padded neighbor slots per dst
# feature permutation: device feature n = c*4 + h  <->  model feature o = h*32 + c
FPERM_O_OF_N = np.array([(n % HEADS) * C + n // HEADS for n in range(H)])
N_CORES = 8
BC = B // N_CORES  # graphs per core


def children(j):
    return [c for c, p in enumerate(PARENTS) if p == j]


def edge_slots():
    """For every real edge (src, dst) return its slot k at dst."""
    slots = {}
    for j in range(J):
        slots[(j, j)] = 0
        p = PARENTS[j]
        if p >= 0:
            slots[(p, j)] = 1
        for i, c in enumerate(children(j)):
            slots[(c, j)] = 2 + i
    return slots


def build_runs():
    """Greedy decomposition of the 70 edges into strided runs.

    Returns list of (src0, sstep, dst0, n, slot). dst steps by +1 within a
    run except the self run (both step 1) and const-src runs (sstep=0).
    """
    slots = edge_slots()
    edges = sorted(slots.keys(), key=lambda e: (slots[e], e[1]))  # by (slot, dst)
    runs = []
    used = set()
    for e in edges:
        if e in used:
            continue
        src, dst = e
        k = slots[e]
        for sstep in (1, 0):
            n = 1
            while True:
                nxt = (src + sstep * n, dst + n)
                if nxt in slots and slots[nxt] == k and nxt not in used:
                    n += 1
                else:
                    break
            if n > 1 or sstep == 0:
                break
        for i in range(n):
            used.add((src + sstep * i, dst + i))
        runs.append((src, sstep, dst, n, k))
    assert sum(r[3] for r in runs) == 70, sum(r[3] for r in runs)
    return runs


def _edges():
    e = []
    for c, p in enumerate(PARENTS):
        if p >= 0:
            e.append((p, c)); e.append((c, p))
    for j in range(len(PARENTS)):
        e.append((j, j))
    a = np.asarray(e, dtype=np.int32)
    return a[:, 0], a[:, 1]


def _ln_np(x, g, b, eps=1e-5):
    m = x.mean(-1, keepdims=True)
    v = x.var(-1, keepdims=True)
    return (x - m) / np.sqrt(v + eps) * g + b


def np_reference(x, p, collect=None):
    """Numpy port of reference.py for an arbitrary batch."""
    Bn = x.shape[0]
    src, dst = _edges()
    h = x @ p["in_w"] + p["in_b"] + p["pos"][None]
    res = x @ p["res_w"] + p["res_b"]
    cc = collect if collect is not None else {}
    for l in range(L):
        hp = h
        xl = (h @ p["gat_w"][l]).reshape(Bn, J, HEADS, C)
        a_s = np.einsum('bjhc,hc->bjh', xl, p["att_s"][l])
        a_d = np.einsum('bjhc,hc->bjh', xl, p["att_d"][l])
        cc[f"xl{l}"] = xl; cc[f"a_s{l}"] = a_s; cc[f"a_d{l}"] = a_d
        e = a_s[:, src] + a_d[:, dst]
        e = np.where(e > 0, e, SLOPE * e)
        ex = np.exp(e)
        den = np.zeros((Bn, J, HEADS), e.dtype)
        np.add.at(den, (slice(None), dst), ex)
        alpha = ex / den[:, dst]
        cc[f"alpha{l}"] = alpha
        msg = xl[:, src] * alpha[..., None]
        out = np.zeros_like(xl)
        np.add.at(out, (slice(None), dst), msg)
        out = out.reshape(Bn, J, H) + p["gat_b"][l]
        cc[f"agg{l}"] = out
        out = np.where(out > 0, out, np.exp(np.minimum(out, 0)) - 1)  # elu
        out = _ln_np(out, p["ln_g"][l], p["ln_b"][l])
        cc[f"o{l}"] = out
        h = out + hp if l > 0 else out
    h = h + res
    y = np.maximum(h @ p["w1"] + p["b1"], 0)
    y = _ln_np(y, p["lng2"], p["lnb2"])
    return y @ p["w2"] + p["b2"]


def host_prep(inputs):
    """All host-side weight algebra."""
    f32 = np.float32
    in_w, in_b = np.asarray(inputs["in_w"]), np.asarray(inputs["in_b"])
    res_w, res_b = np.asarray(inputs["res_w"]), np.asarray(inputs["res_b"])
    pos = np.asarray(inputs["pos"])
    gat_w = np.asarray(inputs["gat_w"])
    att_s, att_d = np.asarray(inputs["att_s"]), np.asarray(inputs["att_d"])
    gat_b = np.asarray(inputs["gat_b"])
    ln_g, ln_b = np.asarray(inputs["ln_g"]), np.asarray(inputs["ln_b"])
    w1, b1 = np.asarray(inputs["w1"]), np.asarray(inputs["b1"])
    lng2, lnb2 = np.asarray(inputs["lng2"]), np.asarray(inputs["lnb2"])
    w2, b2 = np.asarray(inputs["w2"]), np.asarray(inputs["b2"])

    # per-layer score projections: ws[l] [H, 8] cols = (a_s h0..h3, a_d h0..h3)
    ws = np.zeros((L, H, 8), f32)
    for l in range(L):
        for h in range(HEADS):
            ws[l, :, h] = gat_w[l, :, h * C:(h + 1) * C] @ att_s[l, h]
            ws[l, :, 4 + h] = gat_w[l, :, h * C:(h + 1) * C] @ att_d[l, h]

    P = FPERM_O_OF_N

    # L0 folded stationaries on rhs28 = [x(3); ones(1); onehot_j(24)]
    W0p = np.zeros((28, H), f32)
    W0p[0:3] = in_w @ gat_w[0]
    W0p[3] = in_b @ gat_w[0]
    W0p[4:28] = pos @ gat_w[0]
    W0p = W0p[:, P]
    WS0p = np.zeros((28, 8), f32)
    WS0p[0:3] = in_w @ ws[0]
    WS0p[3] = in_b @ ws[0]
    WS0p[4:28] = pos @ ws[0]
    gat_w_d = gat_w[:, P][:, :, P]          # rows+cols permuted (l>=1 use)
    ws_d = ws[:, P]                          # rows permuted
    gat_b_d = gat_b[:, P]
    ln_g_d = ln_g[:, P]
    ln_b_d = ln_b[:, P]
    w1_d = w1[P, :]
    RW1p = np.zeros((28, H // 2), f32)
    RW1p[0:3] = res_w @ w1
    RW1p[3] = res_b @ w1
    # final LN2/w2 fold
    W2p = (lng2[:, None] * w2).astype(f32)          # [64, 6]
    c2 = W2p.sum(axis=0)                             # colsum for -mu*r term
    b2p = lnb2 @ w2 + b2                             # [6]

    # fused [W | ws] stationaries
    Wcat0 = np.concatenate([W0p, WS0p], axis=1)                    # [28, 136]
    Wcat12 = np.concatenate([gat_w_d[1:], ws_d[1:]], axis=2)       # [2, 128, 136]
    return dict(
        Wcat0=Wcat0, Wcat12=Wcat12, RW1p=RW1p, W2p=W2p, c2=c2, b2p=b2p,
        gat_b=gat_b_d, ln_g=ln_g_d, ln_b=ln_b_d, w1=w1_d, b1=b1,
    )


def make_rhs_const(G):
    """Rows 3..27 of rhs28: [ones; onehot_j] as [25, J*G] bf16."""
    import ml_dtypes
    N = J * G
    out = np.zeros((25, N), np.float32)
    out[0] = 1.0
    for j in range(J):
        out[1 + j, j * G:(j + 1) * G] = 1.0
    return out.astype(ml_dtypes.bfloat16)


def make_x_t(x_core, G):
    """x_core [BCk, 24, 3] -> x_t [3, BCk*24] bf16 with col = blk*G*24 + j*G + g."""
    import ml_dtypes
    BCk = x_core.shape[0]
    nblk = BCk // G
    xt = x_core.reshape(nblk, G, J, IN_DIM).transpose(3, 0, 2, 1).reshape(IN_DIM, BCk * J)
    return xt.astype(ml_dtypes.bfloat16)


F32 = mybir.dt.float32
BF16 = mybir.dt.bfloat16
AF = mybir.ActivationFunctionType
ALU = mybir.AluOpType
AX = mybir.AxisListType

RUNS = build_runs()
CHUNK = 512           # final-MLP moving-mode chunk (columns)
MICRO = 128           # token-chunk size (stationary columns)

# fraction of aggregation FD assigned to gpsimd (rest on vector).
# gpsimd TT is ~2.5x slower than vector 2x-mode AND contends with the DVE
# for the shared SBUF port, so keep it near zero.
AGG_G_FRAC = 0.0


def rawap(t, off, dims):
    a = t[:]
    return bass.AP(tensor=a.tensor, offset=a.offset + off,
                   ap=[[a.ap[0][0], a.ap[0][1]]] + [list(d) for d in dims])


def agg_engine_plan():
    """Mults of mid-size runs go to gpsimd (it is idle and they have no
    serial dependencies); all adds stay on vector so the t_v accumulation
    chain never ping-pongs across engines. Tiny runs and the big self run
    stay on vector (gpsimd per-op overhead / 2x-mode advantage)."""
    plan = {}
    for ri, (src0, sstep, dst0, n, k) in enumerate(RUNS):
        fd = n * 2 * 128
        plan[("m", ri)] = "v"
        if ri != 0:
            plan[("a", ri)] = "v"
    return plan


AGG_PLAN = agg_engine_plan()


def kernel_body(ctx, tc, io, G2, n_blocks, flags, dbg_l=None):
    """io: dict name -> bass.AP (dram). Emits the kernel."""
    nc = tc.nc
    G = 128 * G2
    N = J * G
    NB = n_blocks
    n_mc = N // MICRO          # 48 token chunks per block
    jD = G2 * 128              # xl cols per joint in GM
    aD = G2 * HEADS * KMAX
    eD = aD
    GRP = 3                    # psum chunks per bank group
    n_grp = n_mc // GRP        # 16

    triv_ln = flags["triv_ln"]
    triv_gb = flags["triv_gb"]

    x_t, y_out = io["x_t"], io.get("y")

    wp = ctx.enter_context(tc.tile_pool(name="wp", bufs=1))
    consts = ctx.enter_context(tc.tile_pool(name="consts", bufs=1))
    mmp = ctx.enter_context(tc.tile_pool(name="mmp", bufs=4, space="PSUM"))
    psum = ctx.enter_context(tc.tile_pool(name="psum", bufs=2, space="PSUM"))
    psg = ctx.enter_context(tc.tile_pool(name="psg", bufs=2, space="PSUM"))

    # ---- persistent constants ----
    def cload(name, shape, dtype=BF16, src=None):
        t = consts.tile(list(shape), dtype, tag=f"c_{name}")
        nc.sync.dma_start(t[:], src if src is not None else io[name])
        return t

    c_Wc0 = cload("Wcat0", (28, 136))
    c_Wc = [cload(f"Wcat{l}", (H, 136), src=io["Wcat12"][l - 1]) for l in (1, 2)]
    c_W1 = cload("W1", (H, H // 2))
    c_RW1p = cload("RW1p", (28, H // 2))
    c_W2p = cload("W2p", (H // 2, OUT_DIM))
    c_b1 = cload("b1", (H // 2, 1), F32, io["B1"].unsqueeze(1))
    C2B2 = io["C2B2"]
    c_c2 = cload("c2", (128, OUT_DIM), F32,
                 bass.AP(tensor=C2B2.tensor, offset=C2B2.offset,
                         ap=[[0, 128], [1, OUT_DIM]]))
    c_b2p = cload("b2p", (128, OUT_DIM), F32,
                  bass.AP(tensor=C2B2.tensor, offset=C2B2.offset + OUT_DIM,
                          ap=[[0, 128], [1, OUT_DIM]]))
    c_neg1 = consts.tile([128, 1], F32, tag="c_neg1")
    nc.vector.memset(c_neg1[:], -1.0)
    c_lng = c_lnb = None
    if not triv_ln:
        c_lng, c_lnb = [], []
        for l in range(L):
            c_lng.append(cload(f"lng{l}", (H, 1), F32, io["LNG"][l].unsqueeze(1)))
            c_lnb.append(cload(f"lnb{l}", (H, 1), F32, io["LNB"][l].unsqueeze(1)))
    c_gbr = None
    if not triv_gb:
        # gat_b replicated across partitions: [L][128, 128]
        c_gbr = []
        for l in range(L):
            GB = io["GB"]
            src = bass.AP(tensor=GB.tensor, offset=GB.offset + l * H,
                          ap=[[0, 128], [1, H]])
            c_gbr.append(cload(f"gbr{l}", (128, H), F32, src))

    def transpose(dst_t, src_t, eng, q=128):
        eng.dma_start_transpose(
            dst_t[:].rearrange("p (k q) -> p k q", q=q), src_t[:])

    def ln_smalls(ssum, ssq, inv_n):
        """per-token sums -> (sr, sm2) fp32 [128, n_mc] tiles."""
        smu = wp.tile([128, n_mc], F32, tag="smu", bufs=2)
        svar = wp.tile([128, n_mc], F32, tag="svar", bufs=2)
        sr = wp.tile([128, n_mc], F32, tag="sr", bufs=2)
        sm2 = wp.tile([128, n_mc], F32, tag="sm2", bufs=2)
        nc.vector.tensor_scalar_mul(smu[:], ssum[:], inv_n)
        nc.vector.tensor_scalar_mul(svar[:], ssq[:], inv_n)
        nc.vector.tensor_tensor(out=sm2[:], in0=smu[:], in1=smu[:], op=ALU.mult)
        nc.vector.tensor_tensor(out=svar[:], in0=svar[:], in1=sm2[:],
                                op=ALU.subtract)
        nc.vector.tensor_scalar_add(svar[:], svar[:], 1e-5)
        nc.vector.reciprocal_approx_fast(sm2[:], svar[:])
        nc.scalar.activation(sr[:], sm2[:], AF.Sqrt)
        nc.vector.tensor_tensor(out=sm2[:], in0=smu[:], in1=sr[:], op=ALU.mult)
        return sr, sm2

    def block_prog(blk):
        t_h = wp.tile([128, N], BF16, tag="t_h", bufs=2)
        rhs28 = wp.tile([28, N], BF16, tag="rhs28", bufs=2)
        nc.sync.dma_start(rhs28[3:28, :], io["rhs_const"])
        nc.sync.dma_start(rhs28[0:3, :], x_t[:, blk * N:(blk + 1) * N])

        for l in range(L):
            src_t = rhs28 if l == 0 else t_h
            kdim = 28 if l == 0 else H
            wcat = c_Wc0 if l == 0 else c_Wc[l - 1]

            # ---- xl + scores: per-chunk stationary matmuls -> GM ----
            t_xlg = wp.tile([128, N], BF16, tag="xlg", bufs=2)
            sS = wp.tile([128, n_mc * 8], F32, tag="sS", bufs=2)
            for g in range(n_grp):
                pt = mmp.tile([128, 512], F32, tag="mmp")  # one full bank
                for i in range(GRP):
                    mc = g * GRP + i
                    nc.tensor.matmul(
                        pt[:, i * 136:(i + 1) * 136],
                        src_t[0:kdim, mc * MICRO:(mc + 1) * MICRO],
                        wcat[:], start=True, stop=True)
                nc.scalar.activation(
                    t_xlg[:, g * GRP * 128:(g + 1) * GRP * 128],
                    rawap(pt, 0, [(136, GRP), (1, 128)]),
                    AF.Identity)
                nc.scalar.activation(
                    sS[:, g * GRP * 8:(g + 1) * GRP * 8],
                    rawap(pt, 128, [(136, GRP), (1, 8)]),
                    AF.Identity)
            yield
            if dbg_l == l and "dbg_sS" in io:
                nc.sync.dma_start(io["dbg_sS"], sS[:])
            if dbg_l == l and "dbg_xlg" in io:
                nc.sync.dma_start(io["dbg_xlg"], t_xlg[:])

            # ---- E build (gpsimd) ----
            sE = wp.tile([128, J * G2 * HEADS * KMAX], F32, tag="sE", bufs=1)
            nc.gpsimd.memset(sE[:], -10000.0)
            for (src0, sstep, dst0, n, k) in RUNS:
                out_ap = rawap(sE, dst0 * eD + k * HEADS,
                               [(eD, n), (KMAX * HEADS, G2), (1, HEADS)])
                as_ap = rawap(sS, src0 * G2 * 8,
                              [(G2 * 8 * sstep, n), (8, G2), (1, HEADS)])
                ad_ap = rawap(sS, dst0 * G2 * 8 + 4,
                              [(G2 * 8, n), (8, G2), (1, HEADS)])
                nc.gpsimd.tensor_tensor(out=out_ap, in0=as_ap, in1=ad_ap,
                                        op=ALU.add)

            # ---- lrelu (vector) + exp (scalar) ----
            sP = wp.tile([128, J * G2 * HEADS * KMAX], F32, tag="sP", bufs=1)
            nc.vector.tensor_scalar(sP[:], sE[:], 0.0, SLOPE - 1.0,
                                    op0=ALU.min, op1=ALU.mult)
            nc.vector.tensor_tensor(out=sP[:], in0=sE[:], in1=sP[:], op=ALU.add)
            nc.scalar.activation(sP[:], sP[:], AF.Exp)

            # ---- den (vector reduce) + recip + alpha (gpsimd) ----
            sden = wp.tile([128, J * G2 * HEADS], F32, tag="sden", bufs=2)
            sdr = wp.tile([128, J * G2 * HEADS], F32, tag="sdr", bufs=2)
            def pk(k):
                return rawap(sP, k * HEADS,
                             [(eD, J), (KMAX * HEADS, G2), (1, HEADS)])
            den_ap = rawap(sden, 0, [(G2 * HEADS, J), (HEADS, G2), (1, HEADS)])
            nc.vector.tensor_tensor(out=den_ap, in0=pk(0), in1=pk(1),
                                    op=ALU.add)
            for k in range(2, KMAX):
                nc.vector.tensor_tensor(out=den_ap, in0=den_ap, in1=pk(k),
                                        op=ALU.add)
            nc.vector.reciprocal_approx_fast(sdr[:], sden[:])
            sA = wp.tile([128, J * G2 * HEADS * KMAX], BF16, tag="sA", bufs=2)
            nc.gpsimd.tensor_tensor(
                out=rawap(sA, 0, [(eD, J), (KMAX * HEADS, G2),
                                  (HEADS, KMAX), (1, HEADS)]),
                in0=rawap(sP, 0, [(eD, J), (KMAX * HEADS, G2),
                                  (HEADS, KMAX), (1, HEADS)]),
                in1=rawap(sdr, 0, [(G2 * HEADS, J), (HEADS, G2),
                                   (0, KMAX), (1, HEADS)]),
                op=ALU.mult)
            yield
            if dbg_l == l and "dbg_sA" in io:
                nc.sync.dma_start(io["dbg_sA"], sA[:])

            # ---- aggregation (split vector/gpsimd) ----
            t_v = wp.tile([128, N], BF16, tag="t_v", bufs=2)

            def xl_ap(j0, sstep, n):
                return rawap(t_xlg, j0 * jD,
                             [(jD * sstep, n), (128, G2), (1, 128)])

            def al_ap(dst0, n, k):
                return rawap(sA, dst0 * aD + k * HEADS,
                             [(aD, n), (KMAX * HEADS, G2), (0, C),
                              (1, HEADS)])

            def v_ap(dst0, n, buf):
                return rawap(buf, dst0 * jD,
                             [(jD, n), (128, G2), (1, 128)])

            def eng_of(kind, ri):
                return nc.gpsimd if AGG_PLAN[(kind, ri)] == "g" else nc.vector

            max_n = max(r[3] for r in RUNS[1:])
            for ri, (src0, sstep, dst0, n, k) in enumerate(RUNS):
                if ri == 0:
                    eng_of("m", 0).tensor_tensor(
                        out=v_ap(0, 24, t_v), in0=xl_ap(0, 1, 24),
                        in1=al_ap(0, 24, 0), op=ALU.mult)
                    continue
                t_tmp = wp.tile([128, max_n * jD], BF16, tag="atmp", bufs=2)
                tmp_ap = rawap(t_tmp, 0, [(jD, n), (128, G2), (1, 128)])
                eng_of("m", ri).tensor_tensor(
                    out=tmp_ap, in0=xl_ap(src0, sstep, n),
                    in1=al_ap(dst0, n, k), op=ALU.mult)
                eng_of("a", ri).tensor_tensor(
                    out=v_ap(dst0, n, t_v), in0=v_ap(dst0, n, t_v),
                    in1=tmp_ap, op=ALU.add)
            if not triv_gb:
                nc.vector.tensor_tensor(
                    out=t_v[:].rearrange("p (m n) -> p m n", n=128),
                    in0=t_v[:].rearrange("p (m n) -> p m n", n=128),
                    in1=rawap(c_gbr[l], 0, [(0, n_mc), (1, 128)]),
                    op=ALU.add)
            yield
            if dbg_l == l and "dbg_v" in io:
                nc.sync.dma_start(io["dbg_v"], t_v[:])

            # ---- elu: w = relu(v) + expm1(min(v,0)) ----
            # exp in fp32 chunks (bf16 exp output near 1.0 loses the small
            # signal after the -1), expm1 cast on gpsimd.
            t_w = wp.tile([128, N], BF16, tag="t_w", bufs=2)
            t_e = wp.tile([128, N], BF16, tag="t_e", bufs=1)
            NE = 1024
            for c0 in range(0, N, NE):
                e32 = wp.tile([128, NE], F32, tag="e32", bufs=2)
                nc.scalar.activation(e32[:], t_v[:, c0:c0 + NE], AF.Exp)
                nc.vector.tensor_scalar(t_e[:, c0:c0 + NE], e32[:], 1.0, -1.0,
                                        op0=ALU.min, op1=ALU.add)
            nc.vector.tensor_scalar_max(t_w[:], t_v[:], 0.0)
            nc.vector.tensor_tensor(out=t_w[:], in0=t_w[:], in1=t_e[:],
                                    op=ALU.add)
            yield
            if dbg_l == l and "dbg_w" in io:
                nc.sync.dma_start(io["dbg_w"], t_w[:])

            # ---- LN stats: free-dim reduces in GM ----
            ssum = wp.tile([128, n_mc], F32, tag="ssum", bufs=2)
            ssq = wp.tile([128, n_mc], F32, tag="ssq", bufs=2)
            w3 = t_w[:].rearrange("p (m n) -> p m n", n=128)
            nc.vector.tensor_reduce(out=ssum[:], in_=w3, axis=AX.X, op=ALU.add)
            t_sq = t_e  # exp result dead after elu add
            nc.scalar.activation(t_sq[:], t_w[:], AF.Square)
            nc.vector.tensor_reduce(
                out=ssq[:], in_=t_sq[:].rearrange("p (m n) -> p m n", n=128),
                axis=AX.X, op=ALU.add)
            sr, sm2 = ln_smalls(ssum, ssq, 1.0 / H)

            yield
            # ---- o_raw = w*r - mu*r: gpsimd mult, vector subtract ----
            o1 = t_v  # reuse t_v (dead)
            nc.gpsimd.tensor_tensor(
                out=t_w[:].rearrange("p (m n) -> p m n", n=128),
                in0=w3,
                in1=rawap(sr, 0, [(1, n_mc), (0, 128)]),
                op=ALU.mult)
            nc.vector.tensor_tensor(
                out=o1[:].rearrange("p (m n) -> p m n", n=128),
                in0=t_w[:].rearrange("p (m n) -> p m n", n=128),
                in1=rawap(sm2, 0, [(1, n_mc), (0, 128)]),
                op=ALU.subtract)

            # ---- T3 transpose to FM; h update ----
            if triv_ln:
                if l == 0:
                    transpose(t_h, o1, nc.sync)
                else:
                    t_o = t_w  # dead after the subtract
                    transpose(t_o, o1, nc.sync)
                    nc.vector.tensor_tensor(out=t_h[:], in0=t_h[:],
                                            in1=t_o[:], op=ALU.add)
            else:
                t_o = t_w
                transpose(t_o, o1, nc.sync)
                dst_t = t_h if l == 0 else t_o
                nc.scalar.activation(dst_t[:], t_o[:], AF.Identity,
                                     bias=c_lnb[l][:, 0:1],
                                     scale=c_lng[l][:, 0:1])
                if l > 0:
                    nc.vector.tensor_tensor(out=t_h[:], in0=t_h[:],
                                            in1=t_o[:], op=ALU.add)
            if dbg_l == l and "dbg_h" in io:
                nc.sync.dma_start(io["dbg_h"], t_h[:])
            yield

        # ================= final MLP =================
        t_z = wp.tile([64, N], BF16, tag="t_z", bufs=1)
        for c0 in range(0, N, CHUNK):
            pt = psum.tile([128, CHUNK], F32, tag="pt_mm")
            nc.tensor.matmul(pt[0:64, :], c_W1[:], t_h[:, c0:c0 + CHUNK],
                             start=True, stop=False)
            nc.tensor.matmul(pt[0:64, :], c_RW1p[:], rhs28[:, c0:c0 + CHUNK],
                             start=False, stop=True)
            nc.scalar.activation(t_z[:, c0:c0 + CHUNK], pt[0:64, :],
                                 AF.Relu, bias=c_b1[:, 0:1])

        yield
        # LN2 stats via transpose to GM
        z_gm = wp.tile([128, n_mc * 64], BF16, tag="z_gm", bufs=1)
        transpose(z_gm, t_z, nc.scalar, q=64)
        z_sq = wp.tile([128, n_mc * 64], BF16, tag="z_sq", bufs=2)
        nc.gpsimd.tensor_tensor(out=z_sq[:], in0=z_gm[:], in1=z_gm[:],
                                op=ALU.mult)
        ssum2 = wp.tile([128, n_mc], F32, tag="ssum2", bufs=2)
        ssq2 = wp.tile([128, n_mc], F32, tag="ssq2", bufs=2)
        nc.vector.tensor_reduce(
            out=ssum2[:], in_=z_gm[:].rearrange("p (m n) -> p m n", n=64),
            axis=AX.X, op=ALU.add)
        nc.vector.tensor_reduce(
            out=ssq2[:], in_=z_sq[:].rearrange("p (m n) -> p m n", n=64),
            axis=AX.X, op=ALU.add)
        sr2, sm22 = ln_smalls(ssum2, ssq2, 2.0 / H)

        yield
        # y6 = z @ W2p (stationary per chunk)
        st6 = wp.tile([128, n_mc * OUT_DIM], F32, tag="st6", bufs=2)
        for mc0 in range(0, n_mc, 16):
            gn = min(16, n_mc - mc0)
            pm = psg.tile([128, 128], F32, tag="pt_small")
            for i in range(gn):
                mc = mc0 + i
                nc.tensor.matmul(pm[:, i * OUT_DIM:(i + 1) * OUT_DIM],
                                 t_z[:, mc * MICRO:(mc + 1) * MICRO],
                                 c_W2p[:], start=True, stop=True)
            nc.vector.tensor_copy(st6[:, mc0 * OUT_DIM:(mc0 + gn) * OUT_DIM],
                                  pm[:, 0:gn * OUT_DIM])

        sy = wp.tile([128, n_mc * OUT_DIM], F32, tag="sy", bufs=2)
        t6v = st6[:].rearrange("p (m o) -> p m o", o=OUT_DIM)
        yv = sy[:].rearrange("p (m o) -> p m o", o=OUT_DIM)
        rb = rawap(sr2, 0, [(1, n_mc), (0, OUT_DIM)])
        m2b = rawap(sm22, 0, [(1, n_mc), (0, OUT_DIM)])
        c2b = rawap(c_c2, 0, [(0, n_mc), (1, OUT_DIM)])
        b2b = rawap(c_b2p, 0, [(0, n_mc), (1, OUT_DIM)])
        nc.gpsimd.tensor_tensor(out=yv, in0=t6v, in1=rb, op=ALU.mult)
        nc.gpsimd.tensor_tensor(out=t6v, in0=m2b, in1=c2b, op=ALU.mult)
        nc.gpsimd.tensor_tensor(out=yv, in0=yv, in1=t6v, op=ALU.subtract)
        nc.gpsimd.tensor_tensor(out=yv, in0=yv, in1=b2b, op=ALU.add)

        if y_out is None:
            return
        yo = bass.AP(
            tensor=y_out.tensor,
            offset=y_out.offset + blk * G * J * OUT_DIM,
            ap=[[J * OUT_DIM, 128], [OUT_DIM, J], [128 * J * OUT_DIM, G2],
                [1, OUT_DIM]])
        nc.sync.dma_start(yo, sy[:].rearrange("p (j g o) -> p j g o",
                                              j=J, g=G2))

    # drive two blocks in lockstep so every engine has independent work
    PAIR = 2
    for p0 in range(0, NB, PAIR):
        progs = [block_prog(b) for b in range(p0, min(p0 + PAIR, NB))]
        done = [False] * len(progs)
        while not all(done):
            for i, gpr in enumerate(progs):
                if not done[i]:
                    try:
                        next(gpr)
                    except StopIteration:
                        done[i] = True


# ======================================================================
# Host driver: kernel(**inputs) -> np.ndarray
# ======================================================================
G2_FULL = 2
NB_FULL = (B // N_CORES) // (128 * G2_FULL)


def _install_ntff_shim():
    import sys, types
    if "antenv.axon_hooks" in sys.modules:
        return
    mod = types.ModuleType("antenv.axon_hooks")
    mod._hook = None
    mod.set_axon_ntff_profile_hook = lambda h: setattr(mod, "_hook", h)
    mod.get_axon_ntff_profile_hook = lambda: mod._hook
    sys.modules["antenv.axon_hooks"] = mod
    try:
        from trn_agent_boot.trn_boot import _ntff_profile_via_ctypes
        mod.set_axon_ntff_profile_hook(
            _ntff_profile_via_ctypes("/opt/axon/libaxon_pjrt.so"))
    except Exception:
        pass
    try:
        import concourse.bass_utils as bu
        bu.upload_artifacts = lambda tmpdir: tmpdir
    except Exception:
        pass


_NC_CACHE = {}


def _build_nc(G2, NB, flags_key, dbg_l=None):
    key = (G2, NB, flags_key, dbg_l)
    if key in _NC_CACHE:
        return _NC_CACHE[key]
    from contextlib import ExitStack
    from concourse import bacc
    nc = bacc.Bacc("TRN2", target_bir_lowering=False, debug=False,
                   num_devices=N_CORES)
    flags = dict(triv_ln=flags_key[0], triv_gb=flags_key[1])
    G = 128 * G2
    BCk = G * NB
    N = J * G
    n_mc = N // MICRO
    dt = nc.dram_tensor
    io = {}
    def din(name, shape, dtype):
        io[name] = dt(name, shape, dtype, kind="ExternalInput").ap()
    if dbg_l is not None:
        for nm, sh, dty in [
                ("dbg_xlg", (128, N), BF16), ("dbg_sS", (128, n_mc * 8), F32),
                ("dbg_sA", (128, J * G2 * HEADS * KMAX), BF16),
                ("dbg_v", (128, N), BF16), ("dbg_w", (128, N), BF16),
                ("dbg_h", (128, N), BF16)]:
            io[nm] = dt(nm, sh, dty, kind="ExternalOutput").ap()
    din("x_t", (IN_DIM, BCk * J), BF16)
    din("rhs_const", (25, J * G), BF16)
    din("Wcat0", (28, 136), BF16)
    din("Wcat12", (2, H, 136), BF16)
    din("GB", (L, H), F32)
    din("LNG", (L, H), F32)
    din("LNB", (L, H), F32)
    din("W1", (H, H // 2), BF16)
    din("RW1p", (28, H // 2), BF16)
    din("B1", (H // 2,), F32)
    din("W2p", (H // 2, OUT_DIM), BF16)
    din("C2B2", (2, OUT_DIM), F32)
    io["y"] = dt("y", (BCk, J, OUT_DIM), F32, kind="ExternalOutput").ap()
    with tile.TileContext(nc) as tc:
        with ExitStack() as ctx:
            kernel_body(ctx, tc, io, G2, NB, flags, dbg_l=dbg_l)
    nc.compile()
    _NC_CACHE[key] = nc
    return nc


def make_in_maps(inputs, G2=G2_FULL, NB=NB_FULL):
    import ml_dtypes
    bf = ml_dtypes.bfloat16
    pp = host_prep(inputs)
    G = 128 * G2
    BCk = G * NB
    x = np.asarray(inputs["x"], dtype=np.float32)
    shared = dict(
        rhs_const=np.ascontiguousarray(make_rhs_const(G)),
        Wcat0=np.ascontiguousarray(pp["Wcat0"].astype(bf)),
        Wcat12=np.ascontiguousarray(pp["Wcat12"].astype(bf)),
        GB=np.ascontiguousarray(pp["gat_b"].astype(np.float32)),
        LNG=np.ascontiguousarray(pp["ln_g"].astype(np.float32)),
        LNB=np.ascontiguousarray(pp["ln_b"].astype(np.float32)),
        W1=np.ascontiguousarray(pp["w1"].astype(bf)),
        RW1p=np.ascontiguousarray(pp["RW1p"].astype(bf)),
        B1=np.ascontiguousarray(pp["b1"].astype(np.float32)),
        W2p=np.ascontiguousarray(pp["W2p"].astype(bf)),
        C2B2=np.ascontiguousarray(
            np.stack([pp["c2"], pp["b2p"]]).astype(np.float32)),
    )
    in_maps = []
    for core in range(N_CORES):
        xc = x[core * BCk:(core + 1) * BCk]
        m = dict(shared)
        m["x_t"] = np.ascontiguousarray(make_x_t(xc, G))
        in_maps.append(m)
    return in_maps


def _flags_key(inputs):
    ln_g = np.asarray(inputs["ln_g"]); ln_b = np.asarray(inputs["ln_b"])
    gat_b = np.asarray(inputs["gat_b"])
    triv_ln = bool(np.all(ln_g == 1.0) and np.all(ln_b == 0.0))
    triv_gb = bool(np.all(gat_b == 0.0))
    return (triv_ln, triv_gb)


def run_on_cores(inputs, G2=G2_FULL, NB=NB_FULL, trace=False, dbg_l=None):
    _install_ntff_shim()
    from concourse.bass_utils import run_bass_kernel_spmd
    nc = _build_nc(G2, NB, _flags_key(inputs), dbg_l=dbg_l)
    in_maps = make_in_maps(inputs, G2, NB)
    res = run_bass_kernel_spmd(nc, in_maps, list(range(N_CORES)), trace=trace)
    ys = [res.results[c]["y"] for c in range(N_CORES)]
    y = np.concatenate(ys, axis=0).astype(np.float32)
    return y, res


def kernel(**inputs) -> np.ndarray:
    y, _ = run_on_cores(inputs)
    return y


# revision 37
# speedup vs baseline: 1.0003x; 1.0003x over previous
"""Self-contained TRN2 Bass kernel for nn_GATRotationRegressor.

kernel(**inputs) -> [16384, 24, 6] fp32. Data-parallel over 8 NeuronCores;
all layouts/shapes hardcoded for B=16384, J=24, H=128, heads=4, L=3.

v2: GM-native xl+scores via per-chunk stationary matmuls (one transpose per
layer instead of three), LN stats via free-dim reduces in graph-major
layout, elementwise work balanced across Vector/GpSimd/Scalar engines.
"""
from contextlib import ExitStack

import numpy as np

import concourse.bass as bass
import concourse.tile as tile
from concourse import mybir


PARENTS = [-1, 0, 0, 0, 1, 2, 3, 4, 5, 6, 7, 8, 9, 9, 9, 12, 13, 14, 16, 17, 18, 19, 20, 21]
B, J, IN_DIM, H, HEADS, OUT_DIM, L = 16384, 24, 3, 128, 4, 6, 3
C = H // HEADS
SLOPE = 0.2
KMAX = 5           # padded neighbor slots per dst
# feature permutation: device feature n = c*4 + h  <->  model feature o = h*32 + c
FPERM_O_OF_N = np.array([(n % HEADS) * C + n // HEADS for n in range(H)])
N_CORES = 8
BC = B // N_CORES  # graphs per core


def children(j):
    return [c for c, p in enumerate(PARENTS) if p == j]


def edge_slots():
    """For every real edge (src, dst) return its slot k at dst."""
    slots = {}
    for j in range(J):
        slots[(j, j)] = 0
        p = PARENTS[j]
        if p >= 0:
            slots[(p, j)] = 1
        for i, c in enumerate(children(j)):
            slots[(c, j)] = 2 + i
    return slots


def build_runs():
    """Greedy decomposition of the 70 edges into strided runs.

    Returns list of (src0, sstep, dst0, n, slot). dst steps by +1 within a
    run except the self run (both step 1) and const-src runs (sstep=0).
    """
    slots = edge_slots()
    edges = sorted(slots.keys(), key=lambda e: (slots[e], e[1]))  # by (slot, dst)
    runs = []
    used = set()
    for e in edges:
        if e in used:
            continue
        src, dst = e
        k = slots[e]
        for sstep in (1, 0):
            n = 1
            while True:
                nxt = (src + sstep * n, dst + n)
                if nxt in slots and slots[nxt] == k and nxt not in used:
                    n += 1
                else:
                    break
            if n > 1 or sstep == 0:
                break
        for i in range(n):
            used.add((src + sstep * i, dst + i))
        runs.append((src, sstep, dst, n, k))
    assert sum(r[3] for r in runs) == 70, sum(r[3] for r in runs)
    return runs


def _edges():
    e = []
    for c, p in enumerate(PARENTS):
        if p >= 0:
            e.append((p, c)); e.append((c, p))
    for j in range(len(PARENTS)):
        e.append((j, j))
    a = np.asarray(e, dtype=np.int32)
    return a[:, 0], a[:, 1]


def _ln_np(x, g, b, eps=1e-5):
    m = x.mean(-1, keepdims=True)
    v = x.var(-1, keepdims=True)
    return (x - m) / np.sqrt(v + eps) * g + b


def np_reference(x, p, collect=None):
    """Numpy port of reference.py for an arbitrary batch."""
    Bn = x.shape[0]
    src, dst = _edges()
    h = x @ p["in_w"] + p["in_b"] + p["pos"][None]
    res = x @ p["res_w"] + p["res_b"]
    cc = collect if collect is not None else {}
    for l in range(L):
        hp = h
        xl = (h @ p["gat_w"][l]).reshape(Bn, J, HEADS, C)
        a_s = np.einsum('bjhc,hc->bjh', xl, p["att_s"][l])
        a_d = np.einsum('bjhc,hc->bjh', xl, p["att_d"][l])
        cc[f"xl{l}"] = xl; cc[f"a_s{l}"] = a_s; cc[f"a_d{l}"] = a_d
        e = a_s[:, src] + a_d[:, dst]
        e = np.where(e > 0, e, SLOPE * e)
        ex = np.exp(e)
        den = np.zeros((Bn, J, HEADS), e.dtype)
        np.add.at(den, (slice(None), dst), ex)
        alpha = ex / den[:, dst]
        cc[f"alpha{l}"] = alpha
        msg = xl[:, src] * alpha[..., None]
        out = np.zeros_like(xl)
        np.add.at(out, (slice(None), dst), msg)
        out = out.reshape(Bn, J, H) + p["gat_b"][l]
        cc[f"agg{l}"] = out
        out = np.where(out > 0, out, np.exp(np.minimum(out, 0)) - 1)  # elu
        out = _ln_np(out, p["ln_g"][l], p["ln_b"][l])
        cc[f"o{l}"] = out
        h = out + hp if l > 0 else out
    h = h + res
    y = np.maximum(h @ p["w1"] + p["b1"], 0)
    y = _ln_np(y, p["lng2"], p["lnb2"])
    return y @ p["w2"] + p["b2"]


def host_prep(inputs):
    """All host-side weight algebra."""
    f32 = np.float32
    in_w, in_b = np.asarray(inputs["in_w"]), np.asarray(inputs["in_b"])
    res_w, res_b = np.asarray(inputs["res_w"]), np.asarray(inputs["res_b"])
    pos = np.asarray(inputs["pos"])
    gat_w = np.asarray(inputs["gat_w"])
    att_s, att_d = np.asarray(inputs["att_s"]), np.asarray(inputs["att_d"])
    gat_b = np.asarray(inputs["gat_b"])
    ln_g, ln_b = np.asarray(inputs["ln_g"]), np.asarray(inputs["ln_b"])
    w1, b1 = np.asarray(inputs["w1"]), np.asarray(inputs["b1"])
    lng2, lnb2 = np.asarray(inputs["lng2"]), np.asarray(inputs["lnb2"])
    w2, b2 = np.asarray(inputs["w2"]), np.asarray(inputs["b2"])

    # per-layer score projections: ws[l] [H, 8] cols = (a_s h0..h3, a_d h0..h3)
    ws = np.zeros((L, H, 8), f32)
    for l in range(L):
        for h in range(HEADS):
            ws[l, :, h] = gat_w[l, :, h * C:(h + 1) * C] @ att_s[l, h]
            ws[l, :, 4 + h] = gat_w[l, :, h * C:(h + 1) * C] @ att_d[l, h]

    P = FPERM_O_OF_N

    # L0 folded stationaries on rhs28 = [x(3); ones(1); onehot_j(24)]
    W0p = np.zeros((28, H), f32)
    W0p[0:3] = in_w @ gat_w[0]
    W0p[3] = in_b @ gat_w[0]
    W0p[4:28] = pos @ gat_w[0]
    W0p = W0p[:, P]
    WS0p = np.zeros((28, 8), f32)
    WS0p[0:3] = in_w @ ws[0]
    WS0p[3] = in_b @ ws[0]
    WS0p[4:28] = pos @ ws[0]
    gat_w_d = gat_w[:, P][:, :, P]          # rows+cols permuted (l>=1 use)
    ws_d = ws[:, P]                          # rows permuted
    gat_b_d = gat_b[:, P]
    ln_g_d = ln_g[:, P]
    ln_b_d = ln_b[:, P]
    w1_d = w1[P, :]
    RW1p = np.zeros((28, H // 2), f32)
    RW1p[0:3] = res_w @ w1
    RW1p[3] = res_b @ w1
    # final LN2/w2 fold
    W2p = (lng2[:, None] * w2).astype(f32)          # [64, 6]
    c2 = W2p.sum(axis=0)                             # colsum for -mu*r term
    b2p = lnb2 @ w2 + b2                             # [6]

    # fused [W | ws] stationaries
    Wcat0 = np.concatenate([W0p, WS0p], axis=1)                    # [28, 136]
    Wcat12 = np.concatenate([gat_w_d[1:], ws_d[1:]], axis=2)       # [2, 128, 136]
    return dict(
        Wcat0=Wcat0, Wcat12=Wcat12, RW1p=RW1p, W2p=W2p, c2=c2, b2p=b2p,
        gat_b=gat_b_d, ln_g=ln_g_d, ln_b=ln_b_d, w1=w1_d, b1=b1,
    )


def make_rhs_const(G):
    """Rows 3..27 of rhs28: [ones; onehot_j] as [25, J*G] bf16."""
    import ml_dtypes
    N = J * G
    out = np.zeros((25, N), np.float32)
    out[0] = 1.0
    for j in range(J):
        out[1 + j, j * G:(j + 1) * G] = 1.0
    return out.astype(ml_dtypes.bfloat16)


def make_x_t(x_core, G):
    """x_core [BCk, 24, 3] -> x_t [3, BCk*24] bf16 with col = blk*G*24 + j*G + g."""
    import ml_dtypes
    BCk = x_core.shape[0]
    nblk = BCk // G
    xt = x_core.reshape(nblk, G, J, IN_DIM).transpose(3, 0, 2, 1).reshape(IN_DIM, BCk * J)
    return xt.astype(ml_dtypes.bfloat16)


F32 = mybir.dt.float32
BF16 = mybir.dt.bfloat16
AF = mybir.ActivationFunctionType
ALU = mybir.AluOpType
AX = mybir.AxisListType

RUNS = build_runs()
CHUNK = 512           # final-MLP moving-mode chunk (columns)
MICRO = 128           # token-chunk size (stationary columns)

# fraction of aggregation FD assigned to gpsimd (rest on vector).
# gpsimd TT is ~2.5x slower than vector 2x-mode AND contends with the DVE
# for the shared SBUF port, so keep it near zero.
AGG_G_FRAC = 0.0


def rawap(t, off, dims):
    a = t[:]
    return bass.AP(tensor=a.tensor, offset=a.offset + off,
                   ap=[[a.ap[0][0], a.ap[0][1]]] + [list(d) for d in dims])


def agg_engine_plan():
    """Mults of mid-size runs go to gpsimd (it is idle and they have no
    serial dependencies); all adds stay on vector so the t_v accumulation
    chain never ping-pongs across engines. Tiny runs and the big self run
    stay on vector (gpsimd per-op overhead / 2x-mode advantage)."""
    plan = {}
    for ri, (src0, sstep, dst0, n, k) in enumerate(RUNS):
        fd = n * 2 * 128
        plan[("m", ri)] = "v"
        if ri != 0:
            plan[("a", ri)] = "v"
    return plan


AGG_PLAN = agg_engine_plan()


def kernel_body(ctx, tc, io, G2, n_blocks, flags, dbg_l=None):
    """io: dict name -> bass.AP (dram). Emits the kernel."""
    nc = tc.nc
    G = 128 * G2
    N = J * G
    NB = n_blocks
    n_mc = N // MICRO          # 48 token chunks per block
    jD = G2 * 128              # xl cols per joint in GM
    aD = G2 * HEADS * KMAX
    eD = aD
    GRP = 3                    # psum chunks per bank group
    n_grp = n_mc // GRP        # 16

    triv_ln = flags["triv_ln"]
    triv_gb = flags["triv_gb"]

    x_t, y_out = io["x_t"], io.get("y")

    wp = ctx.enter_context(tc.tile_pool(name="wp", bufs=1))
    consts = ctx.enter_context(tc.tile_pool(name="consts", bufs=1))
    mmp = ctx.enter_context(tc.tile_pool(name="mmp", bufs=4, space="PSUM"))
    psum = ctx.enter_context(tc.tile_pool(name="psum", bufs=2, space="PSUM"))
    psg = ctx.enter_context(tc.tile_pool(name="psg", bufs=2, space="PSUM"))

    # ---- persistent constants ----
    def cload(name, shape, dtype=BF16, src=None):
        t = consts.tile(list(shape), dtype, tag=f"c_{name}")
        nc.sync.dma_start(t[:], src if src is not None else io[name])
        return t

    c_Wc0 = cload("Wcat0", (28, 136))
    c_Wc = [cload(f"Wcat{l}", (H, 136), src=io["Wcat12"][l - 1]) for l in (1, 2)]
    c_W1 = cload("W1", (H, H // 2))
    c_RW1p = cload("RW1p", (28, H // 2))
    c_W2p = cload("W2p", (H // 2, OUT_DIM))
    c_b1 = cload("b1", (H // 2, 1), F32, io["B1"].unsqueeze(1))
    C2B2 = io["C2B2"]
    c_c2 = cload("c2", (128, OUT_DIM), F32,
                 bass.AP(tensor=C2B2.tensor, offset=C2B2.offset,
                         ap=[[0, 128], [1, OUT_DIM]]))
    c_b2p = cload("b2p", (128, OUT_DIM), F32,
                  bass.AP(tensor=C2B2.tensor, offset=C2B2.offset + OUT_DIM,
                          ap=[[0, 128], [1, OUT_DIM]]))
    c_neg1 = consts.tile([128, 1], F32, tag="c_neg1")
    nc.vector.memset(c_neg1[:], -1.0)
    c_lng = c_lnb = None
    if not triv_ln:
        c_lng, c_lnb = [], []
        for l in range(L):
            c_lng.append(cload(f"lng{l}", (H, 1), F32, io["LNG"][l].unsqueeze(1)))
            c_lnb.append(cload(f"lnb{l}", (H, 1), F32, io["LNB"][l].unsqueeze(1)))
    c_gbr = None
    if not triv_gb:
        # gat_b replicated across partitions: [L][128, 128]
        c_gbr = []
        for l in range(L):
            GB = io["GB"]
            src = bass.AP(tensor=GB.tensor, offset=GB.offset + l * H,
                          ap=[[0, 128], [1, H]])
            c_gbr.append(cload(f"gbr{l}", (128, H), F32, src))

    def transpose(dst_t, src_t, eng, q=128):
        eng.dma_start_transpose(
            dst_t[:].rearrange("p (k q) -> p k q", q=q), src_t[:])

    def ln_smalls(ssum, ssq, inv_n):
        """per-token sums -> (sr, sm2) fp32 [128, n_mc] tiles."""
        smu = wp.tile([128, n_mc], F32, tag="smu", bufs=2)
        svar = wp.tile([128, n_mc], F32, tag="svar", bufs=2)
        sr = wp.tile([128, n_mc], F32, tag="sr", bufs=2)
        sm2 = wp.tile([128, n_mc], F32, tag="sm2", bufs=2)
        nc.vector.tensor_scalar_mul(smu[:], ssum[:], inv_n)
        nc.vector.tensor_scalar_mul(svar[:], ssq[:], inv_n)
        nc.vector.tensor_tensor(out=sm2[:], in0=smu[:], in1=smu[:], op=ALU.mult)
        nc.vector.tensor_tensor(out=svar[:], in0=svar[:], in1=sm2[:],
                                op=ALU.subtract)
        nc.vector.tensor_scalar_add(svar[:], svar[:], 1e-5)
        nc.vector.reciprocal_approx_fast(sm2[:], svar[:])
        nc.scalar.activation(sr[:], sm2[:], AF.Sqrt)
        nc.vector.tensor_tensor(out=sm2[:], in0=smu[:], in1=sr[:], op=ALU.mult)
        return sr, sm2

    def block_prog(blk):
        t_h = wp.tile([128, N], BF16, tag="t_h", bufs=2)
        rhs28 = wp.tile([28, N], BF16, tag="rhs28", bufs=2)
        nc.sync.dma_start(rhs28[3:28, :], io["rhs_const"])
        nc.sync.dma_start(rhs28[0:3, :], x_t[:, blk * N:(blk + 1) * N])

        for l in range(L):
            src_t = rhs28 if l == 0 else t_h
            kdim = 28 if l == 0 else H
            wcat = c_Wc0 if l == 0 else c_Wc[l - 1]

            # ---- xl + scores: per-chunk stationary matmuls -> GM ----
            t_xlg = wp.tile([128, N], BF16, tag="xlg", bufs=2)
            sS = wp.tile([128, n_mc * 8], F32, tag="sS", bufs=2)
            for g in range(n_grp):
                pt = mmp.tile([128, 512], F32, tag="mmp")  # one full bank
                for i in range(GRP):
                    mc = g * GRP + i
                    nc.tensor.matmul(
                        pt[:, i * 136:(i + 1) * 136],
                        src_t[0:kdim, mc * MICRO:(mc + 1) * MICRO],
                        wcat[:], start=True, stop=True)
                nc.scalar.activation(
                    t_xlg[:, g * GRP * 128:(g + 1) * GRP * 128],
                    rawap(pt, 0, [(136, GRP), (1, 128)]),
                    AF.Identity)
                nc.scalar.activation(
                    sS[:, g * GRP * 8:(g + 1) * GRP * 8],
                    rawap(pt, 128, [(136, GRP), (1, 8)]),
                    AF.Identity)
            yield
            if dbg_l == l and "dbg_sS" in io:
                nc.sync.dma_start(io["dbg_sS"], sS[:])
            if dbg_l == l and "dbg_xlg" in io:
                nc.sync.dma_start(io["dbg_xlg"], t_xlg[:])

            # ---- E build (gpsimd) ----
            sE = wp.tile([128, J * G2 * HEADS * KMAX], F32, tag="sE", bufs=1)
            nc.gpsimd.memset(sE[:], -10000.0)
            for (src0, sstep, dst0, n, k) in RUNS:
                out_ap = rawap(sE, dst0 * eD + k * HEADS,
                               [(eD, n), (KMAX * HEADS, G2), (1, HEADS)])
                as_ap = rawap(sS, src0 * G2 * 8,
                              [(G2 * 8 * sstep, n), (8, G2), (1, HEADS)])
                ad_ap = rawap(sS, dst0 * G2 * 8 + 4,
                              [(G2 * 8, n), (8, G2), (1, HEADS)])
                nc.gpsimd.tensor_tensor(out=out_ap, in0=as_ap, in1=ad_ap,
                                        op=ALU.add)

            # ---- lrelu (vector) + exp (scalar) ----
            sP = wp.tile([128, J * G2 * HEADS * KMAX], F32, tag="sP", bufs=1)
            nc.vector.tensor_scalar(sP[:], sE[:], 0.0, SLOPE - 1.0,
                                    op0=ALU.min, op1=ALU.mult)
            nc.vector.tensor_tensor(out=sP[:], in0=sE[:], in1=sP[:], op=ALU.add)
            nc.scalar.activation(sP[:], sP[:], AF.Exp)

            # ---- den (vector reduce) + recip + alpha (gpsimd) ----
            sden = wp.tile([128, J * G2 * HEADS], F32, tag="sden", bufs=2)
            sdr = wp.tile([128, J * G2 * HEADS], F32, tag="sdr", bufs=2)
            def pk(k):
                return rawap(sP, k * HEADS,
                             [(eD, J), (KMAX * HEADS, G2), (1, HEADS)])
            den_ap = rawap(sden, 0, [(G2 * HEADS, J), (HEADS, G2), (1, HEADS)])
            nc.vector.tensor_tensor(out=den_ap, in0=pk(0), in1=pk(1),
                                    op=ALU.add)
            for k in range(2, KMAX):
                nc.vector.tensor_tensor(out=den_ap, in0=den_ap, in1=pk(k),
                                        op=ALU.add)
            nc.vector.reciprocal_approx_fast(sdr[:], sden[:])
            sA = wp.tile([128, J * G2 * HEADS * KMAX], BF16, tag="sA", bufs=2)
            nc.gpsimd.tensor_tensor(
                out=rawap(sA, 0, [(eD, J), (KMAX * HEADS, G2),
                                  (HEADS, KMAX), (1, HEADS)]),
                in0=rawap(sP, 0, [(eD, J), (KMAX * HEADS, G2),
                                  (HEADS, KMAX), (1, HEADS)]),
                in1=rawap(sdr, 0, [(G2 * HEADS, J), (HEADS, G2),
                                   (0, KMAX), (1, HEADS)]),
                op=ALU.mult)
            yield
            if dbg_l == l and "dbg_sA" in io:
                nc.sync.dma_start(io["dbg_sA"], sA[:])

            # ---- aggregation (split vector/gpsimd) ----
            t_v = wp.tile([128, N], BF16, tag="t_v", bufs=2)

            def xl_ap(j0, sstep, n):
                return rawap(t_xlg, j0 * jD,
                             [(jD * sstep, n), (128, G2), (1, 128)])

            def al_ap(dst0, n, k):
                return rawap(sA, dst0 * aD + k * HEADS,
                             [(aD, n), (KMAX * HEADS, G2), (0, C),
                              (1, HEADS)])

            def v_ap(dst0, n, buf):
                return rawap(buf, dst0 * jD,
                             [(jD, n), (128, G2), (1, 128)])

            def eng_of(kind, ri):
                return nc.gpsimd if AGG_PLAN[(kind, ri)] == "g" else nc.vector

            max_n = max(r[3] for r in RUNS[1:])
            for ri, (src0, sstep, dst0, n, k) in enumerate(RUNS):
                if ri == 0:
                    eng_of("m", 0).tensor_tensor(
                        out=v_ap(0, 24, t_v), in0=xl_ap(0, 1, 24),
                        in1=al_ap(0, 24, 0), op=ALU.mult)
                    continue
                t_tmp = wp.tile([128, max_n * jD], BF16, tag="atmp", bufs=2)
                tmp_ap = rawap(t_tmp, 0, [(jD, n), (128, G2), (1, 128)])
                eng_of("m", ri).tensor_tensor(
                    out=tmp_ap, in0=xl_ap(src0, sstep, n),
                    in1=al_ap(dst0, n, k), op=ALU.mult)
                eng_of("a", ri).tensor_tensor(
                    out=v_ap(dst0, n, t_v), in0=v_ap(dst0, n, t_v),
                    in1=tmp_ap, op=ALU.add)
            if not triv_gb:
                nc.vector.tensor_tensor(
                    out=t_v[:].rearrange("p (m n) -> p m n", n=128),
                    in0=t_v[:].rearrange("p (m n) -> p m n", n=128),
                    in1=rawap(c_gbr[l], 0, [(0, n_mc), (1, 128)]),
                    op=ALU.add)
            yield
            if dbg_l == l and "dbg_v" in io:
                nc.sync.dma_start(io["dbg_v"], t_v[:])

            # ---- elu: w = relu(v) + expm1(min(v,0)) ----
            # exp in fp32 chunks (bf16 exp output near 1.0 loses the small
            # signal after the -1), expm1 cast on gpsimd.
            t_w = wp.tile([128, N], BF16, tag="t_w", bufs=2)
            t_e = wp.tile([128, N], BF16, tag="t_e", bufs=1)
            NE = 1024
            for c0 in range(0, N, NE):
                e32 = wp.tile([128, NE], F32, tag="e32", bufs=2)
                nc.scalar.activation(e32[:], t_v[:, c0:c0 + NE], AF.Exp)
                nc.vector.tensor_scalar(t_e[:, c0:c0 + NE], e32[:], 1.0, -1.0,
                                        op0=ALU.min, op1=ALU.add)
            nc.vector.tensor_scalar_max(t_w[:], t_v[:], 0.0)
            nc.vector.tensor_tensor(out=t_w[:], in0=t_w[:], in1=t_e[:],
                                    op=ALU.add)
            yield
            if dbg_l == l and "dbg_w" in io:
                nc.sync.dma_start(io["dbg_w"], t_w[:])

            # ---- LN stats: free-dim reduces in GM ----
            ssum = wp.tile([128, n_mc], F32, tag="ssum", bufs=2)
            ssq = wp.tile([128, n_mc], F32, tag="ssq", bufs=2)
            w3 = t_w[:].rearrange("p (m n) -> p m n", n=128)
            nc.vector.tensor_reduce(out=ssum[:], in_=w3, axis=AX.X, op=ALU.add)
            # per-group Square with accumulate: scalar engine computes the
            # squares AND the per-token sums, freeing the vector reduce
            for mc in range(n_mc):
                nc.scalar.activation(
                    t_e[:, mc * 128:(mc + 1) * 128],
                    t_w[:, mc * 128:(mc + 1) * 128],
                    AF.Square, accum_out=ssq[:, mc:mc + 1])
            sr, sm2 = ln_smalls(ssum, ssq, 1.0 / H)

            yield
            # ---- o_raw = w*r - mu*r: fused per-chunk TS (scalar pair) ----
            o1 = t_v  # reuse t_v (dead)
            for mc in range(n_mc):
                nc.vector.tensor_scalar(
                    o1[:, mc * 128:(mc + 1) * 128],
                    t_w[:, mc * 128:(mc + 1) * 128],
                    sr[:, mc:mc + 1], sm2[:, mc:mc + 1],
                    op0=ALU.mult, op1=ALU.subtract)

            # ---- T3 transpose to FM; h update ----
            if triv_ln:
                if l == 0:
                    transpose(t_h, o1, nc.sync)
                else:
                    t_o = t_w  # dead after the subtract
                    transpose(t_o, o1, nc.sync)
                    nc.vector.tensor_tensor(out=t_h[:], in0=t_h[:],
                                            in1=t_o[:], op=ALU.add)
            else:
                t_o = t_w
                transpose(t_o, o1, nc.sync)
                dst_t = t_h if l == 0 else t_o
                nc.scalar.activation(dst_t[:], t_o[:], AF.Identity,
                                     bias=c_lnb[l][:, 0:1],
                                     scale=c_lng[l][:, 0:1])
                if l > 0:
                    nc.vector.tensor_tensor(out=t_h[:], in0=t_h[:],
                                            in1=t_o[:], op=ALU.add)
            if dbg_l == l and "dbg_h" in io:
                nc.sync.dma_start(io["dbg_h"], t_h[:])
            yield

        # ================= final MLP =================
        t_z = wp.tile([64, N], BF16, tag="t_z", bufs=1)
        for c0 in range(0, N, CHUNK):
            pt = psum.tile([128, CHUNK], F32, tag="pt_mm")
            nc.tensor.matmul(pt[0:64, :], c_W1[:], t_h[:, c0:c0 + CHUNK],
                             start=True, stop=False)
            nc.tensor.matmul(pt[0:64, :], c_RW1p[:], rhs28[:, c0:c0 + CHUNK],
                             start=False, stop=True)
            nc.scalar.activation(t_z[:, c0:c0 + CHUNK], pt[0:64, :],
                                 AF.Relu, bias=c_b1[:, 0:1])

        yield
        # LN2 stats via transpose to GM
        z_gm = wp.tile([128, n_mc * 64], BF16, tag="z_gm", bufs=1)
        transpose(z_gm, t_z, nc.scalar, q=64)
        z_sq = wp.tile([128, n_mc * 64], BF16, tag="z_sq", bufs=2)
        nc.gpsimd.tensor_tensor(out=z_sq[:], in0=z_gm[:], in1=z_gm[:],
                                op=ALU.mult)
        ssum2 = wp.tile([128, n_mc], F32, tag="ssum2", bufs=2)
        ssq2 = wp.tile([128, n_mc], F32, tag="ssq2", bufs=2)
        nc.vector.tensor_reduce(
            out=ssum2[:], in_=z_gm[:].rearrange("p (m n) -> p m n", n=64),
            axis=AX.X, op=ALU.add)
        nc.vector.tensor_reduce(
            out=ssq2[:], in_=z_sq[:].rearrange("p (m n) -> p m n", n=64),
            axis=AX.X, op=ALU.add)
        sr2, sm22 = ln_smalls(ssum2, ssq2, 2.0 / H)

        yield
        # y6 = z @ W2p (stationary per chunk)
        st6 = wp.tile([128, n_mc * OUT_DIM], F32, tag="st6", bufs=2)
        for mc0 in range(0, n_mc, 16):
            gn = min(16, n_mc - mc0)
            pm = psg.tile([128, 128], F32, tag="pt_small")
            for i in range(gn):
                mc = mc0 + i
                nc.tensor.matmul(pm[:, i * OUT_DIM:(i + 1) * OUT_DIM],
                                 t_z[:, mc * MICRO:(mc + 1) * MICRO],
                                 c_W2p[:], start=True, stop=True)
            nc.vector.tensor_copy(st6[:, mc0 * OUT_DIM:(mc0 + gn) * OUT_DIM],
                                  pm[:, 0:gn * OUT_DIM])

        sy = wp.tile([128, n_mc * OUT_DIM], F32, tag="sy", bufs=2)
        t6v = st6[:].rearrange("p (m o) -> p m o", o=OUT_DIM)
        yv = sy[:].rearrange("p (m o) -> p m o", o=OUT_DIM)
        rb = rawap(sr2, 0, [(1, n_mc), (0, OUT_DIM)])
        m2b = rawap(sm22, 0, [(1, n_mc), (0, OUT_DIM)])
        c2b = rawap(c_c2, 0, [(0, n_mc), (1, OUT_DIM)])
        b2b = rawap(c_b2p, 0, [(0, n_mc), (1, OUT_DIM)])
        nc.gpsimd.tensor_tensor(out=yv, in0=t6v, in1=rb, op=ALU.mult)
        nc.gpsimd.tensor_tensor(out=t6v, in0=m2b, in1=c2b, op=ALU.mult)
        nc.gpsimd.tensor_tensor(out=yv, in0=yv, in1=t6v, op=ALU.subtract)
        nc.gpsimd.tensor_tensor(out=yv, in0=yv, in1=b2b, op=ALU.add)

        if y_out is None:
            return
        yo = bass.AP(
            tensor=y_out.tensor,
            offset=y_out.offset + blk * G * J * OUT_DIM,
            ap=[[J * OUT_DIM, 128], [OUT_DIM, J], [128 * J * OUT_DIM, G2],
                [1, OUT_DIM]])
        nc.sync.dma_start(yo, sy[:].rearrange("p (j g o) -> p j g o",
                                              j=J, g=G2))

    # drive two blocks in lockstep so every engine has independent work
    PAIR = 2
    for p0 in range(0, NB, PAIR):
        progs = [block_prog(b) for b in range(p0, min(p0 + PAIR, NB))]
        done = [False] * len(progs)
        while not all(done):
            for i, gpr in enumerate(progs):
                if not done[i]:
                    try:
                        next(gpr)
                    except StopIteration:
                        done[i] = True


# ======================================================================
# Host driver: kernel(**inputs) -> np.ndarray
# ======================================================================
G2_FULL = 2
NB_FULL = (B // N_CORES) // (128 * G2_FULL)


def _install_ntff_shim():
    import sys, types
    if "antenv.axon_hooks" in sys.modules:
        return
    mod = types.ModuleType("antenv.axon_hooks")
    mod._hook = None
    mod.set_axon_ntff_profile_hook = lambda h: setattr(mod, "_hook", h)
    mod.get_axon_ntff_profile_hook = lambda: mod._hook
    sys.modules["antenv.axon_hooks"] = mod
    try:
        from trn_agent_boot.trn_boot import _ntff_profile_via_ctypes
        mod.set_axon_ntff_profile_hook(
            _ntff_profile_via_ctypes("/opt/axon/libaxon_pjrt.so"))
    except Exception:
        pass
    try:
        import concourse.bass_utils as bu
        bu.upload_artifacts = lambda tmpdir: tmpdir
    except Exception:
        pass


_NC_CACHE = {}


def _build_nc(G2, NB, flags_key, dbg_l=None):
    key = (G2, NB, flags_key, dbg_l)
    if key in _NC_CACHE:
        return _NC_CACHE[key]
    from contextlib import ExitStack
    from concourse import bacc
    nc = bacc.Bacc("TRN2", target_bir_lowering=False, debug=False,
                   num_devices=N_CORES)
    flags = dict(triv_ln=flags_key[0], triv_gb=flags_key[1])
    G = 128 * G2
    BCk = G * NB
    N = J * G
    n_mc = N // MICRO
    dt = nc.dram_tensor
    io = {}
    def din(name, shape, dtype):
        io[name] = dt(name, shape, dtype, kind="ExternalInput").ap()
    if dbg_l is not None:
        for nm, sh, dty in [
                ("dbg_xlg", (128, N), BF16), ("dbg_sS", (128, n_mc * 8), F32),
                ("dbg_sA", (128, J * G2 * HEADS * KMAX), BF16),
                ("dbg_v", (128, N), BF16), ("dbg_w", (128, N), BF16),
                ("dbg_h", (128, N), BF16)]:
            io[nm] = dt(nm, sh, dty, kind="ExternalOutput").ap()
    din("x_t", (IN_DIM, BCk * J), BF16)
    din("rhs_const", (25, J * G), BF16)
    din("Wcat0", (28, 136), BF16)
    din("Wcat12", (2, H, 136), BF16)
    din("GB", (L, H), F32)
    din("LNG", (L, H), F32)
    din("LNB", (L, H), F32)
    din("W1", (H, H // 2), BF16)
    din("RW1p", (28, H // 2), BF16)
    din("B1", (H // 2,), F32)
    din("W2p", (H // 2, OUT_DIM), BF16)
    din("C2B2", (2, OUT_DIM), F32)
    io["y"] = dt("y", (BCk, J, OUT_DIM), F32, kind="ExternalOutput").ap()
    with tile.TileContext(nc) as tc:
        with ExitStack() as ctx:
            kernel_body(ctx, tc, io, G2, NB, flags, dbg_l=dbg_l)
    nc.compile()
    _NC_CACHE[key] = nc
    return nc


def make_in_maps(inputs, G2=G2_FULL, NB=NB_FULL):
    import ml_dtypes
    bf = ml_dtypes.bfloat16
    pp = host_prep(inputs)
    G = 128 * G2
    BCk = G * NB
    x = np.asarray(inputs["x"], dtype=np.float32)
    shared = dict(
        rhs_const=np.ascontiguousarray(make_rhs_const(G)),
        Wcat0=np.ascontiguousarray(pp["Wcat0"].astype(bf)),
        Wcat12=np.ascontiguousarray(pp["Wcat12"].astype(bf)),
        GB=np.ascontiguousarray(pp["gat_b"].astype(np.float32)),
        LNG=np.ascontiguousarray(pp["ln_g"].astype(np.float32)),
        LNB=np.ascontiguousarray(pp["ln_b"].astype(np.float32)),
        W1=np.ascontiguousarray(pp["w1"].astype(bf)),
        RW1p=np.ascontiguousarray(pp["RW1p"].astype(bf)),
        B1=np.ascontiguousarray(pp["b1"].astype(np.float32)),
        W2p=np.ascontiguousarray(pp["W2p"].astype(bf)),
        C2B2=np.ascontiguousarray(
            np.stack([pp["c2"], pp["b2p"]]).astype(np.float32)),
    )
    in_maps = []
    for core in range(N_CORES):
        xc = x[core * BCk:(core + 1) * BCk]
        m = dict(shared)
        m["x_t"] = np.ascontiguousarray(make_x_t(xc, G))
        in_maps.append(m)
    return in_maps


def _flags_key(inputs):
    ln_g = np.asarray(inputs["ln_g"]); ln_b = np.asarray(inputs["ln_b"])
    gat_b = np.asarray(inputs["gat_b"])
    triv_ln = bool(np.all(ln_g == 1.0) and np.all(ln_b == 0.0))
    triv_gb = bool(np.all(gat_b == 0.0))
    return (triv_ln, triv_gb)


def run_on_cores(inputs, G2=G2_FULL, NB=NB_FULL, trace=False, dbg_l=None):
    _install_ntff_shim()
    from concourse.bass_utils import run_bass_kernel_spmd
    nc = _build_nc(G2, NB, _flags_key(inputs), dbg_l=dbg_l)
    in_maps = make_in_maps(inputs, G2, NB)
    res = run_bass_kernel_spmd(nc, in_maps, list(range(N_CORES)), trace=trace)
    ys = [res.results[c]["y"] for c in range(N_CORES)]
    y = np.concatenate(ys, axis=0).astype(np.float32)
    return y, res


def kernel(**inputs) -> np.ndarray:
    y, _ = run_on_cores(inputs)
    return y


# revision 38
# speedup vs baseline: 1.0749x; 1.0746x over previous
"""Self-contained TRN2 Bass kernel for nn_GATRotationRegressor.

kernel(**inputs) -> [16384, 24, 6] fp32. Data-parallel over 8 NeuronCores;
all layouts/shapes hardcoded for B=16384, J=24, H=128, heads=4, L=3.

v2: GM-native xl+scores via per-chunk stationary matmuls (one transpose per
layer instead of three), LN stats via free-dim reduces in graph-major
layout, elementwise work balanced across Vector/GpSimd/Scalar engines.
"""
from contextlib import ExitStack

import numpy as np

import concourse.bass as bass
import concourse.tile as tile
from concourse import mybir


PARENTS = [-1, 0, 0, 0, 1, 2, 3, 4, 5, 6, 7, 8, 9, 9, 9, 12, 13, 14, 16, 17, 18, 19, 20, 21]
B, J, IN_DIM, H, HEADS, OUT_DIM, L = 16384, 24, 3, 128, 4, 6, 3
C = H // HEADS
SLOPE = 0.2
KMAX = 5           # padded neighbor slots per dst
# feature permutation: device feature n = c*4 + h  <->  model feature o = h*32 + c
FPERM_O_OF_N = np.array([(n % HEADS) * C + n // HEADS for n in range(H)])
N_CORES = 8
BC = B // N_CORES  # graphs per core


def children(j):
    return [c for c, p in enumerate(PARENTS) if p == j]


def edge_slots():
    """For every real edge (src, dst) return its slot k at dst."""
    slots = {}
    for j in range(J):
        slots[(j, j)] = 0
        p = PARENTS[j]
        if p >= 0:
            slots[(p, j)] = 1
        for i, c in enumerate(children(j)):
            slots[(c, j)] = 2 + i
    return slots


def build_runs():
    """Greedy decomposition of the 70 edges into strided runs.

    Returns list of (src0, sstep, dst0, n, slot). dst steps by +1 within a
    run except the self run (both step 1) and const-src runs (sstep=0).
    """
    slots = edge_slots()
    edges = sorted(slots.keys(), key=lambda e: (slots[e], e[1]))  # by (slot, dst)
    runs = []
    used = set()
    for e in edges:
        if e in used:
            continue
        src, dst = e
        k = slots[e]
        for sstep in (1, 0):
            n = 1
            while True:
                nxt = (src + sstep * n, dst + n)
                if nxt in slots and slots[nxt] == k and nxt not in used:
                    n += 1
                else:
                    break
            if n > 1 or sstep == 0:
                break
        for i in range(n):
            used.add((src + sstep * i, dst + i))
        runs.append((src, sstep, dst, n, k))
    assert sum(r[3] for r in runs) == 70, sum(r[3] for r in runs)
    return runs


def _edges():
    e = []
    for c, p in enumerate(PARENTS):
        if p >= 0:
            e.append((p, c)); e.append((c, p))
    for j in range(len(PARENTS)):
        e.append((j, j))
    a = np.asarray(e, dtype=np.int32)
    return a[:, 0], a[:, 1]


def _ln_np(x, g, b, eps=1e-5):
    m = x.mean(-1, keepdims=True)
    v = x.var(-1, keepdims=True)
    return (x - m) / np.sqrt(v + eps) * g + b


def np_reference(x, p, collect=None):
    """Numpy port of reference.py for an arbitrary batch."""
    Bn = x.shape[0]
    src, dst = _edges()
    h = x @ p["in_w"] + p["in_b"] + p["pos"][None]
    res = x @ p["res_w"] + p["res_b"]
    cc = collect if collect is not None else {}
    for l in range(L):
        hp = h
        xl = (h @ p["gat_w"][l]).reshape(Bn, J, HEADS, C)
        a_s = np.einsum('bjhc,hc->bjh', xl, p["att_s"][l])
        a_d = np.einsum('bjhc,hc->bjh', xl, p["att_d"][l])
        cc[f"xl{l}"] = xl; cc[f"a_s{l}"] = a_s; cc[f"a_d{l}"] = a_d
        e = a_s[:, src] + a_d[:, dst]
        e = np.where(e > 0, e, SLOPE * e)
        ex = np.exp(e)
        den = np.zeros((Bn, J, HEADS), e.dtype)
        np.add.at(den, (slice(None), dst), ex)
        alpha = ex / den[:, dst]
        cc[f"alpha{l}"] = alpha
        msg = xl[:, src] * alpha[..., None]
        out = np.zeros_like(xl)
        np.add.at(out, (slice(None), dst), msg)
        out = out.reshape(Bn, J, H) + p["gat_b"][l]
        cc[f"agg{l}"] = out
        out = np.where(out > 0, out, np.exp(np.minimum(out, 0)) - 1)  # elu
        out = _ln_np(out, p["ln_g"][l], p["ln_b"][l])
        cc[f"o{l}"] = out
        h = out + hp if l > 0 else out
    h = h + res
    y = np.maximum(h @ p["w1"] + p["b1"], 0)
    y = _ln_np(y, p["lng2"], p["lnb2"])
    return y @ p["w2"] + p["b2"]


def host_prep(inputs):
    """All host-side weight algebra."""
    f32 = np.float32
    in_w, in_b = np.asarray(inputs["in_w"]), np.asarray(inputs["in_b"])
    res_w, res_b = np.asarray(inputs["res_w"]), np.asarray(inputs["res_b"])
    pos = np.asarray(inputs["pos"])
    gat_w = np.asarray(inputs["gat_w"])
    att_s, att_d = np.asarray(inputs["att_s"]), np.asarray(inputs["att_d"])
    gat_b = np.asarray(inputs["gat_b"])
    ln_g, ln_b = np.asarray(inputs["ln_g"]), np.asarray(inputs["ln_b"])
    w1, b1 = np.asarray(inputs["w1"]), np.asarray(inputs["b1"])
    lng2, lnb2 = np.asarray(inputs["lng2"]), np.asarray(inputs["lnb2"])
    w2, b2 = np.asarray(inputs["w2"]), np.asarray(inputs["b2"])

    # per-layer score projections: ws[l] [H, 8] cols = (a_s h0..h3, a_d h0..h3)
    ws = np.zeros((L, H, 8), f32)
    for l in range(L):
        for h in range(HEADS):
            ws[l, :, h] = gat_w[l, :, h * C:(h + 1) * C] @ att_s[l, h]
            ws[l, :, 4 + h] = gat_w[l, :, h * C:(h + 1) * C] @ att_d[l, h]

    P = FPERM_O_OF_N

    # L0 folded stationaries on rhs28 = [x(3); ones(1); onehot_j(24)]
    W0p = np.zeros((28, H), f32)
    W0p[0:3] = in_w @ gat_w[0]
    W0p[3] = in_b @ gat_w[0]
    W0p[4:28] = pos @ gat_w[0]
    W0p = W0p[:, P]
    WS0p = np.zeros((28, 8), f32)
    WS0p[0:3] = in_w @ ws[0]
    WS0p[3] = in_b @ ws[0]
    WS0p[4:28] = pos @ ws[0]
    gat_w_d = gat_w[:, P][:, :, P]          # rows+cols permuted (l>=1 use)
    ws_d = ws[:, P]                          # rows permuted
    gat_b_d = gat_b[:, P]
    ln_g_d = ln_g[:, P]
    ln_b_d = ln_b[:, P]
    w1_d = w1[P, :]
    RW1p = np.zeros((28, H // 2), f32)
    RW1p[0:3] = res_w @ w1
    RW1p[3] = res_b @ w1
    # final LN2/w2 fold
    W2p = (lng2[:, None] * w2).astype(f32)          # [64, 6]
    c2 = W2p.sum(axis=0)                             # colsum for -mu*r term
    b2p = lnb2 @ w2 + b2                             # [6]

    # fused [W | ws] stationaries
    Wcat0 = np.concatenate([W0p, WS0p], axis=1)                    # [28, 136]
    Wcat12 = np.concatenate([gat_w_d[1:], ws_d[1:]], axis=2)       # [2, 128, 136]
    return dict(
        Wcat0=Wcat0, Wcat12=Wcat12, RW1p=RW1p, W2p=W2p, c2=c2, b2p=b2p,
        gat_b=gat_b_d, ln_g=ln_g_d, ln_b=ln_b_d, w1=w1_d, b1=b1,
    )


def make_rhs_const(G):
    """Rows 3..27 of rhs28: [ones; onehot_j] as [25, J*G] bf16."""
    import ml_dtypes
    N = J * G
    out = np.zeros((25, N), np.float32)
    out[0] = 1.0
    for j in range(J):
        out[1 + j, j * G:(j + 1) * G] = 1.0
    return out.astype(ml_dtypes.bfloat16)


def make_x_t(x_core, G):
    """x_core [BCk, 24, 3] -> x_t [3, BCk*24] bf16 with col = blk*G*24 + j*G + g."""
    import ml_dtypes
    BCk = x_core.shape[0]
    nblk = BCk // G
    xt = x_core.reshape(nblk, G, J, IN_DIM).transpose(3, 0, 2, 1).reshape(IN_DIM, BCk * J)
    return xt.astype(ml_dtypes.bfloat16)


F32 = mybir.dt.float32
BF16 = mybir.dt.bfloat16
AF = mybir.ActivationFunctionType
ALU = mybir.AluOpType
AX = mybir.AxisListType

RUNS = build_runs()
CHUNK = 512           # final-MLP moving-mode chunk (columns)
MICRO = 128           # token-chunk size (stationary columns)

# fraction of aggregation FD assigned to gpsimd (rest on vector).
# gpsimd TT is ~2.5x slower than vector 2x-mode AND contends with the DVE
# for the shared SBUF port, so keep it near zero.
AGG_G_FRAC = 0.0


def rawap(t, off, dims):
    a = t[:]
    return bass.AP(tensor=a.tensor, offset=a.offset + off,
                   ap=[[a.ap[0][0], a.ap[0][1]]] + [list(d) for d in dims])


def agg_engine_plan():
    """Mults of mid-size runs go to gpsimd (it is idle and they have no
    serial dependencies); all adds stay on vector so the t_v accumulation
    chain never ping-pongs across engines. Tiny runs and the big self run
    stay on vector (gpsimd per-op overhead / 2x-mode advantage)."""
    plan = {}
    for ri, (src0, sstep, dst0, n, k) in enumerate(RUNS):
        fd = n * 2 * 128
        plan[("m", ri)] = "v"
        if ri != 0:
            plan[("a", ri)] = "v"
    return plan


AGG_PLAN = agg_engine_plan()


def kernel_body(ctx, tc, io, G2, n_blocks, flags, dbg_l=None):
    """io: dict name -> bass.AP (dram). Emits the kernel."""
    nc = tc.nc
    G = 128 * G2
    N = J * G
    NB = n_blocks
    n_mc = N // MICRO          # 48 token chunks per block
    jD = G2 * 128              # xl cols per joint in GM
    aD = G2 * HEADS * KMAX
    eD = aD
    GRP = 3                    # psum chunks per bank group
    n_grp = n_mc // GRP        # 16

    triv_ln = flags["triv_ln"]
    triv_gb = flags["triv_gb"]

    x_t, y_out = io["x_t"], io.get("y")

    wp = ctx.enter_context(tc.tile_pool(name="wp", bufs=1))
    consts = ctx.enter_context(tc.tile_pool(name="consts", bufs=1))
    mmp = ctx.enter_context(tc.tile_pool(name="mmp", bufs=4, space="PSUM"))
    psum = ctx.enter_context(tc.tile_pool(name="psum", bufs=2, space="PSUM"))
    psg = ctx.enter_context(tc.tile_pool(name="psg", bufs=2, space="PSUM"))

    # ---- persistent constants ----
    def cload(name, shape, dtype=BF16, src=None):
        t = consts.tile(list(shape), dtype, tag=f"c_{name}")
        nc.sync.dma_start(t[:], src if src is not None else io[name])
        return t

    c_Wc0 = cload("Wcat0", (28, 136))
    c_Wc = [cload(f"Wcat{l}", (H, 136), src=io["Wcat12"][l - 1]) for l in (1, 2)]
    c_W1 = cload("W1", (H, H // 2))
    c_RW1p = cload("RW1p", (28, H // 2))
    c_W2p = cload("W2p", (H // 2, OUT_DIM))
    c_b1 = cload("b1", (H // 2, 1), F32, io["B1"].unsqueeze(1))
    C2B2 = io["C2B2"]
    c_c2 = cload("c2", (128, OUT_DIM), F32,
                 bass.AP(tensor=C2B2.tensor, offset=C2B2.offset,
                         ap=[[0, 128], [1, OUT_DIM]]))
    c_b2p = cload("b2p", (128, OUT_DIM), F32,
                  bass.AP(tensor=C2B2.tensor, offset=C2B2.offset + OUT_DIM,
                          ap=[[0, 128], [1, OUT_DIM]]))
    c_neg1 = consts.tile([128, 1], F32, tag="c_neg1")
    nc.vector.memset(c_neg1[:], -1.0)
    c_lng = c_lnb = None
    if not triv_ln:
        c_lng, c_lnb = [], []
        for l in range(L):
            c_lng.append(cload(f"lng{l}", (H, 1), F32, io["LNG"][l].unsqueeze(1)))
            c_lnb.append(cload(f"lnb{l}", (H, 1), F32, io["LNB"][l].unsqueeze(1)))
    c_gbr = None
    if not triv_gb:
        # gat_b replicated across partitions: [L][128, 128]
        c_gbr = []
        for l in range(L):
            GB = io["GB"]
            src = bass.AP(tensor=GB.tensor, offset=GB.offset + l * H,
                          ap=[[0, 128], [1, H]])
            c_gbr.append(cload(f"gbr{l}", (128, H), F32, src))

    def transpose(dst_t, src_t, eng, q=128):
        eng.dma_start_transpose(
            dst_t[:].rearrange("p (k q) -> p k q", q=q), src_t[:])

    def ln_smalls(ssum, ssq, inv_n):
        """per-token sums -> (sr, sm2) fp32 [128, n_mc] tiles."""
        smu = wp.tile([128, n_mc], F32, tag="smu", bufs=2)
        svar = wp.tile([128, n_mc], F32, tag="svar", bufs=2)
        sr = wp.tile([128, n_mc], F32, tag="sr", bufs=2)
        sm2 = wp.tile([128, n_mc], F32, tag="sm2", bufs=2)
        nc.vector.tensor_scalar_mul(smu[:], ssum[:], inv_n)
        nc.vector.tensor_scalar_mul(svar[:], ssq[:], inv_n)
        nc.vector.tensor_tensor(out=sm2[:], in0=smu[:], in1=smu[:], op=ALU.mult)
        nc.vector.tensor_tensor(out=svar[:], in0=svar[:], in1=sm2[:],
                                op=ALU.subtract)
        nc.vector.tensor_scalar_add(svar[:], svar[:], 1e-5)
        nc.vector.reciprocal_approx_fast(sm2[:], svar[:])
        nc.scalar.activation(sr[:], sm2[:], AF.Sqrt)
        nc.vector.tensor_tensor(out=sm2[:], in0=smu[:], in1=sr[:], op=ALU.mult)
        return sr, sm2

    def block_prog(blk):
        t_h = wp.tile([128, N], BF16, tag="t_h", bufs=2)
        rhs28 = wp.tile([28, N], BF16, tag="rhs28", bufs=2)
        nc.sync.dma_start(rhs28[3:28, :], io["rhs_const"])
        nc.sync.dma_start(rhs28[0:3, :], x_t[:, blk * N:(blk + 1) * N])

        for l in range(L):
            src_t = rhs28 if l == 0 else t_h
            kdim = 28 if l == 0 else H
            wcat = c_Wc0 if l == 0 else c_Wc[l - 1]

            # ---- xl + scores: per-chunk stationary matmuls -> GM ----
            t_xlg = wp.tile([128, N], BF16, tag="xlg", bufs=2)
            sS = wp.tile([128, n_mc * 8], F32, tag="sS", bufs=2)
            for g in range(n_grp):
                pt = mmp.tile([128, 512], F32, tag="mmp")  # one full bank
                for i in range(GRP):
                    mc = g * GRP + i
                    nc.tensor.matmul(
                        pt[:, i * 136:(i + 1) * 136],
                        src_t[0:kdim, mc * MICRO:(mc + 1) * MICRO],
                        wcat[:], start=True, stop=True)
                nc.scalar.activation(
                    t_xlg[:, g * GRP * 128:(g + 1) * GRP * 128],
                    rawap(pt, 0, [(136, GRP), (1, 128)]),
                    AF.Identity)
                nc.scalar.activation(
                    sS[:, g * GRP * 8:(g + 1) * GRP * 8],
                    rawap(pt, 128, [(136, GRP), (1, 8)]),
                    AF.Identity)
            yield
            if dbg_l == l and "dbg_sS" in io:
                nc.sync.dma_start(io["dbg_sS"], sS[:])
            if dbg_l == l and "dbg_xlg" in io:
                nc.sync.dma_start(io["dbg_xlg"], t_xlg[:])

            # ---- E build (gpsimd) ----
            sE = wp.tile([128, J * G2 * HEADS * KMAX], F32, tag="sE", bufs=1)
            nc.gpsimd.memset(sE[:], -10000.0)
            for (src0, sstep, dst0, n, k) in RUNS:
                out_ap = rawap(sE, dst0 * eD + k * HEADS,
                               [(eD, n), (KMAX * HEADS, G2), (1, HEADS)])
                as_ap = rawap(sS, src0 * G2 * 8,
                              [(G2 * 8 * sstep, n), (8, G2), (1, HEADS)])
                ad_ap = rawap(sS, dst0 * G2 * 8 + 4,
                              [(G2 * 8, n), (8, G2), (1, HEADS)])
                nc.gpsimd.tensor_tensor(out=out_ap, in0=as_ap, in1=ad_ap,
                                        op=ALU.add)

            # ---- lrelu (vector) + exp (scalar) ----
            sP = wp.tile([128, J * G2 * HEADS * KMAX], F32, tag="sP", bufs=1)
            nc.vector.tensor_scalar(sP[:], sE[:], 0.0, SLOPE - 1.0,
                                    op0=ALU.min, op1=ALU.mult)
            nc.vector.tensor_tensor(out=sP[:], in0=sE[:], in1=sP[:], op=ALU.add)
            nc.scalar.activation(sP[:], sP[:], AF.Exp)

            # ---- den (vector reduce) + recip + alpha (gpsimd) ----
            sden = wp.tile([128, J * G2 * HEADS], F32, tag="sden", bufs=2)
            sdr = wp.tile([128, J * G2 * HEADS], F32, tag="sdr", bufs=2)
            def pk(k):
                return rawap(sP, k * HEADS,
                             [(eD, J), (KMAX * HEADS, G2), (1, HEADS)])
            den_ap = rawap(sden, 0, [(G2 * HEADS, J), (HEADS, G2), (1, HEADS)])
            nc.vector.tensor_tensor(out=den_ap, in0=pk(0), in1=pk(1),
                                    op=ALU.add)
            for k in range(2, KMAX):
                nc.vector.tensor_tensor(out=den_ap, in0=den_ap, in1=pk(k),
                                        op=ALU.add)
            nc.vector.reciprocal_approx_fast(sdr[:], sden[:])
            sA = wp.tile([128, J * G2 * HEADS * KMAX], BF16, tag="sA", bufs=2)
            nc.gpsimd.tensor_tensor(
                out=rawap(sA, 0, [(eD, J), (KMAX * HEADS, G2),
                                  (HEADS, KMAX), (1, HEADS)]),
                in0=rawap(sP, 0, [(eD, J), (KMAX * HEADS, G2),
                                  (HEADS, KMAX), (1, HEADS)]),
                in1=rawap(sdr, 0, [(G2 * HEADS, J), (HEADS, G2),
                                   (0, KMAX), (1, HEADS)]),
                op=ALU.mult)
            yield
            if dbg_l == l and "dbg_sA" in io:
                nc.sync.dma_start(io["dbg_sA"], sA[:])

            # ---- aggregation (split vector/gpsimd) ----
            t_v = wp.tile([128, N], BF16, tag="t_v", bufs=2)

            def xl_ap(j0, sstep, n):
                return rawap(t_xlg, j0 * jD,
                             [(jD * sstep, n), (128, G2), (1, 128)])

            def al_ap(dst0, n, k):
                return rawap(sA, dst0 * aD + k * HEADS,
                             [(aD, n), (KMAX * HEADS, G2), (0, C),
                              (1, HEADS)])

            def v_ap(dst0, n, buf):
                return rawap(buf, dst0 * jD,
                             [(jD, n), (128, G2), (1, 128)])

            def eng_of(kind, ri):
                return nc.gpsimd if AGG_PLAN[(kind, ri)] == "g" else nc.vector

            max_n = max(r[3] for r in RUNS[1:])
            for ri, (src0, sstep, dst0, n, k) in enumerate(RUNS):
                if ri == 0:
                    eng_of("m", 0).tensor_tensor(
                        out=v_ap(0, 24, t_v), in0=xl_ap(0, 1, 24),
                        in1=al_ap(0, 24, 0), op=ALU.mult)
                    continue
                t_tmp = wp.tile([128, max_n * jD], BF16, tag="atmp", bufs=2)
                tmp_ap = rawap(t_tmp, 0, [(jD, n), (128, G2), (1, 128)])
                eng_of("m", ri).tensor_tensor(
                    out=tmp_ap, in0=xl_ap(src0, sstep, n),
                    in1=al_ap(dst0, n, k), op=ALU.mult)
                eng_of("a", ri).tensor_tensor(
                    out=v_ap(dst0, n, t_v), in0=v_ap(dst0, n, t_v),
                    in1=tmp_ap, op=ALU.add)
            if not triv_gb:
                nc.vector.tensor_tensor(
                    out=t_v[:].rearrange("p (m n) -> p m n", n=128),
                    in0=t_v[:].rearrange("p (m n) -> p m n", n=128),
                    in1=rawap(c_gbr[l], 0, [(0, n_mc), (1, 128)]),
                    op=ALU.add)
            yield
            if dbg_l == l and "dbg_v" in io:
                nc.sync.dma_start(io["dbg_v"], t_v[:])

            # ---- elu: w = relu(v) + expm1(min(v,0)) ----
            # exp in fp32 chunks (bf16 exp output near 1.0 loses the small
            # signal after the -1), expm1 cast on gpsimd.
            t_w = wp.tile([128, N], BF16, tag="t_w", bufs=2)
            t_e = wp.tile([128, N], BF16, tag="t_e", bufs=1)
            NE = 1024
            for c0 in range(0, N, NE):
                e32 = wp.tile([128, NE], F32, tag="e32", bufs=2)
                nc.scalar.activation(e32[:], t_v[:, c0:c0 + NE], AF.Exp)
                nc.vector.tensor_scalar(t_e[:, c0:c0 + NE], e32[:], 1.0, -1.0,
                                        op0=ALU.min, op1=ALU.add)
            nc.vector.tensor_scalar_max(t_w[:], t_v[:], 0.0)
            nc.vector.tensor_tensor(out=t_w[:], in0=t_w[:], in1=t_e[:],
                                    op=ALU.add)
            yield
            if dbg_l == l and "dbg_w" in io:
                nc.sync.dma_start(io["dbg_w"], t_w[:])

            # ---- LN stats: free-dim reduces in GM ----
            ssum = wp.tile([128, n_mc], F32, tag="ssum", bufs=2)
            ssq = wp.tile([128, n_mc], F32, tag="ssq", bufs=2)
            w3 = t_w[:].rearrange("p (m n) -> p m n", n=128)
            nc.vector.tensor_reduce(out=ssum[:], in_=w3, axis=AX.X, op=ALU.add)
            t_sq = t_e  # exp result dead after elu add
            nc.scalar.activation(t_sq[:], t_w[:], AF.Square)
            nc.vector.tensor_reduce(
                out=ssq[:], in_=t_sq[:].rearrange("p (m n) -> p m n", n=128),
                axis=AX.X, op=ALU.add)
            sr, sm2 = ln_smalls(ssum, ssq, 1.0 / H)

            yield
            # ---- o_raw = w*r - mu*r: fused per-chunk TS (scalar pair) ----
            o1 = t_v  # reuse t_v (dead)
            for mc in range(n_mc):
                nc.vector.tensor_scalar(
                    o1[:, mc * 128:(mc + 1) * 128],
                    t_w[:, mc * 128:(mc + 1) * 128],
                    sr[:, mc:mc + 1], sm2[:, mc:mc + 1],
                    op0=ALU.mult, op1=ALU.subtract)

            # ---- T3 transpose to FM; h update ----
            if triv_ln:
                if l == 0:
                    transpose(t_h, o1, nc.sync)
                else:
                    t_o = t_w  # dead after the subtract
                    transpose(t_o, o1, nc.sync)
                    nc.vector.tensor_tensor(out=t_h[:], in0=t_h[:],
                                            in1=t_o[:], op=ALU.add)
            else:
                t_o = t_w
                transpose(t_o, o1, nc.sync)
                dst_t = t_h if l == 0 else t_o
                nc.scalar.activation(dst_t[:], t_o[:], AF.Identity,
                                     bias=c_lnb[l][:, 0:1],
                                     scale=c_lng[l][:, 0:1])
                if l > 0:
                    nc.vector.tensor_tensor(out=t_h[:], in0=t_h[:],
                                            in1=t_o[:], op=ALU.add)
            if dbg_l == l and "dbg_h" in io:
                nc.sync.dma_start(io["dbg_h"], t_h[:])
            yield

        # ================= final MLP =================
        t_z = wp.tile([64, N], BF16, tag="t_z", bufs=1)
        for c0 in range(0, N, CHUNK):
            pt = psum.tile([128, CHUNK], F32, tag="pt_mm")
            nc.tensor.matmul(pt[0:64, :], c_W1[:], t_h[:, c0:c0 + CHUNK],
                             start=True, stop=False)
            nc.tensor.matmul(pt[0:64, :], c_RW1p[:], rhs28[:, c0:c0 + CHUNK],
                             start=False, stop=True)
            nc.scalar.activation(t_z[:, c0:c0 + CHUNK], pt[0:64, :],
                                 AF.Relu, bias=c_b1[:, 0:1])

        yield
        # LN2 stats via transpose to GM
        z_gm = wp.tile([128, n_mc * 64], BF16, tag="z_gm", bufs=1)
        transpose(z_gm, t_z, nc.scalar, q=64)
        z_sq = wp.tile([128, n_mc * 64], BF16, tag="z_sq", bufs=2)
        nc.gpsimd.tensor_tensor(out=z_sq[:], in0=z_gm[:], in1=z_gm[:],
                                op=ALU.mult)
        ssum2 = wp.tile([128, n_mc], F32, tag="ssum2", bufs=2)
        ssq2 = wp.tile([128, n_mc], F32, tag="ssq2", bufs=2)
        nc.vector.tensor_reduce(
            out=ssum2[:], in_=z_gm[:].rearrange("p (m n) -> p m n", n=64),
            axis=AX.X, op=ALU.add)
        nc.vector.tensor_reduce(
            out=ssq2[:], in_=z_sq[:].rearrange("p (m n) -> p m n", n=64),
            axis=AX.X, op=ALU.add)
        sr2, sm22 = ln_smalls(ssum2, ssq2, 2.0 / H)

        yield
        # y6 = z @ W2p (stationary per chunk)
        st6 = wp.tile([128, n_mc * OUT_DIM], F32, tag="st6", bufs=2)
        for mc0 in range(0, n_mc, 16):
            gn = min(16, n_mc - mc0)
            pm = psg.tile([128, 128], F32, tag="pt_small")
            for i in range(gn):
                mc = mc0 + i
                nc.tensor.matmul(pm[:, i * OUT_DIM:(i + 1) * OUT_DIM],
                                 t_z[:, mc * MICRO:(mc + 1) * MICRO],
                                 c_W2p[:], start=True, stop=True)
            nc.vector.tensor_copy(st6[:, mc0 * OUT_DIM:(mc0 + gn) * OUT_DIM],
                                  pm[:, 0:gn * OUT_DIM])

        sy = wp.tile([128, n_mc * OUT_DIM], F32, tag="sy", bufs=2)
        t6v = st6[:].rearrange("p (m o) -> p m o", o=OUT_DIM)
        yv = sy[:].rearrange("p (m o) -> p m o", o=OUT_DIM)
        rb = rawap(sr2, 0, [(1, n_mc), (0, OUT_DIM)])
        m2b = rawap(sm22, 0, [(1, n_mc), (0, OUT_DIM)])
        c2b = rawap(c_c2, 0, [(0, n_mc), (1, OUT_DIM)])
        b2b = rawap(c_b2p, 0, [(0, n_mc), (1, OUT_DIM)])
        nc.gpsimd.tensor_tensor(out=yv, in0=t6v, in1=rb, op=ALU.mult)
        nc.gpsimd.tensor_tensor(out=t6v, in0=m2b, in1=c2b, op=ALU.mult)
        nc.gpsimd.tensor_tensor(out=yv, in0=yv, in1=t6v, op=ALU.subtract)
        nc.gpsimd.tensor_tensor(out=yv, in0=yv, in1=b2b, op=ALU.add)

        if y_out is None:
            return
        yo = bass.AP(
            tensor=y_out.tensor,
            offset=y_out.offset + blk * G * J * OUT_DIM,
            ap=[[J * OUT_DIM, 128], [OUT_DIM, J], [128 * J * OUT_DIM, G2],
                [1, OUT_DIM]])
        nc.sync.dma_start(yo, sy[:].rearrange("p (j g o) -> p j g o",
                                              j=J, g=G2))

    # drive two blocks in lockstep so every engine has independent work
    PAIR = 2
    for p0 in range(0, NB, PAIR):
        progs = [block_prog(b) for b in range(p0, min(p0 + PAIR, NB))]
        done = [False] * len(progs)
        while not all(done):
            for i, gpr in enumerate(progs):
                if not done[i]:
                    try:
                        next(gpr)
                    except StopIteration:
                        done[i] = True


# ======================================================================
# Host driver: kernel(**inputs) -> np.ndarray
# ======================================================================
G2_FULL = 2
NB_FULL = (B // N_CORES) // (128 * G2_FULL)


def _install_ntff_shim():
    import sys, types
    if "antenv.axon_hooks" in sys.modules:
        return
    mod = types.ModuleType("antenv.axon_hooks")
    mod._hook = None
    mod.set_axon_ntff_profile_hook = lambda h: setattr(mod, "_hook", h)
    mod.get_axon_ntff_profile_hook = lambda: mod._hook
    sys.modules["antenv.axon_hooks"] = mod
    try:
        from trn_agent_boot.trn_boot import _ntff_profile_via_ctypes
        mod.set_axon_ntff_profile_hook(
            _ntff_profile_via_ctypes("/opt/axon/libaxon_pjrt.so"))
    except Exception:
        pass
    try:
        import concourse.bass_utils as bu
        bu.upload_artifacts = lambda tmpdir: tmpdir
    except Exception:
        pass


_NC_CACHE = {}


def _build_nc(G2, NB, flags_key, dbg_l=None):
    key = (G2, NB, flags_key, dbg_l)
    if key in _NC_CACHE:
        return _NC_CACHE[key]
    from contextlib import ExitStack
    from concourse import bacc
    nc = bacc.Bacc("TRN2", target_bir_lowering=False, debug=False,
                   num_devices=N_CORES)
    flags = dict(triv_ln=flags_key[0], triv_gb=flags_key[1])
    G = 128 * G2
    BCk = G * NB
    N = J * G
    n_mc = N // MICRO
    dt = nc.dram_tensor
    io = {}
    def din(name, shape, dtype):
        io[name] = dt(name, shape, dtype, kind="ExternalInput").ap()
    if dbg_l is not None:
        for nm, sh, dty in [
                ("dbg_xlg", (128, N), BF16), ("dbg_sS", (128, n_mc * 8), F32),
                ("dbg_sA", (128, J * G2 * HEADS * KMAX), BF16),
                ("dbg_v", (128, N), BF16), ("dbg_w", (128, N), BF16),
                ("dbg_h", (128, N), BF16)]:
            io[nm] = dt(nm, sh, dty, kind="ExternalOutput").ap()
    din("x_t", (IN_DIM, BCk * J), BF16)
    din("rhs_const", (25, J * G), BF16)
    din("Wcat0", (28, 136), BF16)
    din("Wcat12", (2, H, 136), BF16)
    din("GB", (L, H), F32)
    din("LNG", (L, H), F32)
    din("LNB", (L, H), F32)
    din("W1", (H, H // 2), BF16)
    din("RW1p", (28, H // 2), BF16)
    din("B1", (H // 2,), F32)
    din("W2p", (H // 2, OUT_DIM), BF16)
    din("C2B2", (2, OUT_DIM), F32)
    io["y"] = dt("y", (BCk, J, OUT_DIM), F32, kind="ExternalOutput").ap()
    with tile.TileContext(nc) as tc:
        with ExitStack() as ctx:
            kernel_body(ctx, tc, io, G2, NB, flags, dbg_l=dbg_l)
    nc.compile()
    _NC_CACHE[key] = nc
    return nc


def make_in_maps(inputs, G2=G2_FULL, NB=NB_FULL):
    import ml_dtypes
    bf = ml_dtypes.bfloat16
    pp = host_prep(inputs)
    G = 128 * G2
    BCk = G * NB
    x = np.asarray(inputs["x"], dtype=np.float32)
    shared = dict(
        rhs_const=np.ascontiguousarray(make_rhs_const(G)),
        Wcat0=np.ascontiguousarray(pp["Wcat0"].astype(bf)),
        Wcat12=np.ascontiguousarray(pp["Wcat12"].astype(bf)),
        GB=np.ascontiguousarray(pp["gat_b"].astype(np.float32)),
        LNG=np.ascontiguousarray(pp["ln_g"].astype(np.float32)),
        LNB=np.ascontiguousarray(pp["ln_b"].astype(np.float32)),
        W1=np.ascontiguousarray(pp["w1"].astype(bf)),
        RW1p=np.ascontiguousarray(pp["RW1p"].astype(bf)),
        B1=np.ascontiguousarray(pp["b1"].astype(np.float32)),
        W2p=np.ascontiguousarray(pp["W2p"].astype(bf)),
        C2B2=np.ascontiguousarray(
            np.stack([pp["c2"], pp["b2p"]]).astype(np.float32)),
    )
    in_maps = []
    for core in range(N_CORES):
        xc = x[core * BCk:(core + 1) * BCk]
        m = dict(shared)
        m["x_t"] = np.ascontiguousarray(make_x_t(xc, G))
        in_maps.append(m)
    return in_maps


def _flags_key(inputs):
    ln_g = np.asarray(inputs["ln_g"]); ln_b = np.asarray(inputs["ln_b"])
    gat_b = np.asarray(inputs["gat_b"])
    triv_ln = bool(np.all(ln_g == 1.0) and np.all(ln_b == 0.0))
    triv_gb = bool(np.all(gat_b == 0.0))
    return (triv_ln, triv_gb)


def run_on_cores(inputs, G2=G2_FULL, NB=NB_FULL, trace=False, dbg_l=None):
    _install_ntff_shim()
    from concourse.bass_utils import run_bass_kernel_spmd
    nc = _build_nc(G2, NB, _flags_key(inputs), dbg_l=dbg_l)
    in_maps = make_in_maps(inputs, G2, NB)
    res = run_bass_kernel_spmd(nc, in_maps, list(range(N_CORES)), trace=trace)
    ys = [res.results[c]["y"] for c in range(N_CORES)]
    y = np.concatenate(ys, axis=0).astype(np.float32)
    return y, res


def kernel(**inputs) -> np.ndarray:
    y, _ = run_on_cores(inputs)
    return y
